# revision 1
# baseline (speedup 1.0000x reference)
"""Trainium2 Bass kernel for a 6-layer 4D CNN (3^4 SAME convs + PReLU).

Problem: x (8, 2, 16,16,16,16) -> 6 conv layers, channels 2->32->32->32->32
->32->2, PReLU (scalar slope) after the first five convs.

Strategy (per NeuronCore, data-parallel over batch N=8 across 8 cores):
  * d-axis banded-Toeplitz matmuls: activations live in SBUF in a
    "decimated" layout X'[32*s + ch, (a, b, c_pad, j)] where block s in 0..3
    holds d = 2*j + s - 1 (d-phases).  One matmul then contracts
    K = 128 = (4 d-phases x 32 ch) and produces M = 64 = (2 d-outs x 32 ch)
    outputs per column: the 3-tap d-convolution is folded into the
    stationary (block-banded) weight matrix.
  * (da, db, dc) taps: 27 PSUM-accumulated matmuls with shifted windows;
    c is zero-padded in the layout (no clipping), b clipped via windows,
    a via skip.
  * 2-way column packing (tile_position (0,0) / (0,64)) runs two spatial
    chunks concurrently on the 128x128 PE array.
  * Layer 0 (2 input channels) uses an a-partition scheme: partitions =
    (ch, a), M = 128 = (4 a-outs x 32 ch), a-banded stationaries; the dd
    taps are folded into K = 96 via three d-shifted input replicas, so the
    27 taps collapse to 9 (db, dc) matmul groups.  (Concurrent row-group
    tile_position matmuls accumulating into one PSUM bank fault on HW —
    single full-K matmuls are used throughout.)
  * Layer 5 (2 output channels) uses M = 4 = (2 ch x 2 d-outs) with 4-way
    column packing; result leaves in decimated layout, host reassembles.
  * PReLU(x) = max(x, slope*x) via one scalar_tensor_tensor op, fused with
    the psum->SBUF eviction; fp16 activations, fp32 PSUM accumulation.
"""

import sys

import numpy as np

for _p in ("/opt/trn_rl_repo", "/root/.axon_site/_ro/trn_rl_repo"):
    if _p not in sys.path:
        sys.path.append(_p)

import concourse.bass as bass  # noqa: E402
import concourse.mybir as mybir  # noqa: E402
import concourse.tile as tile  # noqa: E402
from concourse import bacc, bass_utils  # noqa: E402
from concourse._compat import with_exitstack  # noqa: E402

F32 = mybir.dt.float32
F16 = mybir.dt.float16

LB = 16
CP = 18   # padded c axis (c in -1..16)
DP = 18   # padded d axis in x_pad
J = 8     # d//2

# tap orderings (db major with db=0 first so the first matmul of every psum
# chunk covers the full window and can carry start=True)
G_MID = [(da, db, dc)
         for db in (0, -1, 1) for da in (0, -1, 1) for dc in (0, -1, 1)]
# L0 groups: (db, dc) only — the dd taps are folded into K=96 via three
# d-shifted partition-block replicas of the input (block rg holds x shifted
# by dd = rg - 1)
G_L0 = [(db, dc) for db in (0, -1, 1) for dc in (0, -1, 1)]


def _pack_weights(ks, la):
    """Host-side packing of conv kernels into stationary matrices (fp16)."""
    na4 = la // 4
    k0, k1, k2, k3, k4, k5 = [np.asarray(k, np.float32) for k in ks]

    # L0: W0[32*rg + i*la + a_in, (g*na4 + a0b)*128 + a_j*32 + o]
    # K = 96 = 3 d-shift blocks (rg -> dd = rg - 1) x (2 ch x la a_in, banded)
    w0 = np.zeros((128, len(G_L0) * na4 * 128), np.float32)
    for gi, (db, dc) in enumerate(G_L0):
        for a0b in range(na4):
            cb = (gi * na4 + a0b) * 128
            for rg in range(3):
                for aj in range(4):
                    for da in (-1, 0, 1):
                        ain = a0b * 4 + aj + da
                        if not (0 <= ain < la):
                            continue
                        for i in range(2):
                            w0[32 * rg + i * la + ain,
                               cb + aj * 32:cb + aj * 32 + 32] = \
                                k0[:, i, da + 1, db + 1, dc + 1, rg]

    # mid layers: W[32*s + i, g*64 + t*32 + o]
    def pack_mid(k):
        w = np.zeros((128, 27 * 64), np.float32)
        for gi, (da, db, dc) in enumerate(G_MID):
            for s in range(4):
                for t in range(2):
                    if 0 <= s - t <= 2:
                        w[32 * s:32 * s + 32, gi * 64 + t * 32:gi * 64 + t * 32 + 32] = \
                            k[:, :, da + 1, db + 1, dc + 1, s - t].T
        return w

    # L5: W5[32*s + i, g*4 + o*2 + t]
    w5 = np.zeros((128, 27 * 4), np.float32)
    for gi, (da, db, dc) in enumerate(G_MID):
        for s in range(4):
            for t in range(2):
                if 0 <= s - t <= 2:
                    for o in range(2):
                        w5[32 * s:32 * s + 32, gi * 4 + o * 2 + t] = \
                            k5[o, :, da + 1, db + 1, dc + 1, s - t]

    return ([w0.astype(np.float16)] +
            [pack_mid(k).astype(np.float16) for k in (k1, k2, k3, k4)] +
            [w5.astype(np.float16)])


@with_exitstack
def _conv_kernel(ctx, tc, la, slopes):
    """Emit the full 6-layer conv program. slopes: python floats len 5."""
    nc = tc.nc
    na4 = la // 4
    xcols = la * LB * CP * J
    pcols = LB * CP * DP

    xin = nc.dram_tensor("xin", [2 * la, 4096], F32, kind="ExternalInput")
    w0_d = nc.dram_tensor("w0", [128, len(G_L0) * na4 * 128],
                          F16, kind="ExternalInput")
    wmid_d = [nc.dram_tensor(f"w{l}", [128, 27 * 64], F16,
                             kind="ExternalInput") for l in (1, 2, 3, 4)]
    w5_d = nc.dram_tensor("w5", [128, 27 * 4], F16, kind="ExternalInput")
    out_d = nc.dram_tensor("out_dec", [4, la * 2048], F32,
                           kind="ExternalOutput")

    const = ctx.enter_context(tc.tile_pool(name="const", bufs=1))
    pp = ctx.enter_context(tc.tile_pool(name="ps", bufs=8, space="PSUM"))
    stg = ctx.enter_context(tc.tile_pool(name="stg", bufs=4))

    # ---- load weights ----
    w0t = const.tile([128, w0_d.shape[1]], F16)
    nc.sync.dma_start(w0t[:], w0_d[:])
    wmt = []
    for wd in wmid_d:
        t = const.tile([128, 27 * 64], F16, tag=wd.name)
        nc.sync.dma_start(t[:], wd[:])
        wmt.append(t)
    w5t = const.tile([128, 27 * 4], F16)
    nc.sync.dma_start(w5t[:], w5_d[:])

    # ---- build x_pad [128, (b, c_pad, d_pad)] fp16, replicated x4 ----
    xsb = const.tile([2 * la, 4096], F32)
    nc.sync.dma_start(xsb[:], xin[:])
    xpad = const.tile([128, pcols], F16)
    nc.vector.memset(xpad[:], 0.0)
    xp4 = xpad.rearrange("p (b c d) -> p b c d", b=LB, c=CP, d=DP)
    xs4 = xsb.rearrange("p (b c d) -> p b c d", b=LB, c=16, d=16)
    # block rg holds x shifted in d by dd = rg - 1 (zero-padded edges)
    for rg in range(3):
        dst = xp4[32 * rg:32 * rg + 2 * la, :, 1:17, 2 - rg:18 - rg]
        if rg == 1:
            nc.vector.tensor_copy(dst, xs4[:])
        else:
            nc.scalar.copy(dst, xs4[:])

    # ---- X' ping-pong buffers ----
    xa = const.tile([128, xcols], F16)
    xb = const.tile([128, xcols], F16)
    nc.gpsimd.memset(xa[:], 0.0)
    nc.gpsimd.memset(xb[:], 0.0)
    bufs = [xa, xb]

    def xview(t):
        return t.rearrange("p (a b c j) -> p a b c j", a=la, b=LB, c=CP, j=J)

    # scatter map: s -> (j_lo, j_cnt, d_lo)  [d = 2j + s - 1]
    SMAP = {0: (1, 7, 1), 1: (0, 8, 0), 2: (0, 8, 1), 3: (0, 7, 2)}

    # ================= layer 0 =================
    xn4 = xview(bufs[0])
    a_slope = slopes[0]
    for a0b in range(na4):
        for bc in range(8):          # b-pair chunks
            b0 = bc * 2
            ps = pp.tile([128, 512], F32, tag="ps")
            p4 = ps.rearrange("p (b c d) -> p b c d", b=2, c=16, d=16)
            for gi, (db, dc) in enumerate(G_L0):
                blo = max(b0, -db)
                bhi = min(b0 + 2, 16 - db)
                cb = (gi * na4 + a0b) * 128
                # K = 96: three d-shift blocks; rhs d-slice 1:17 uniform
                rhs = xp4[0:96, blo + db:bhi + db, dc + 1:dc + 17, 1:17]
                out = p4[:, blo - b0:bhi - b0, :, :]
                nc.tensor.matmul(out, w0t[0:96, cb:cb + 128], rhs,
                                 start=(gi == 0),
                                 stop=(gi == len(G_L0) - 1))
            # prelu the whole chunk into fp16 staging, then scatter
            sg = stg.tile([128, 512], F16, tag="l0st")
            nc.scalar.activation(sg[:], ps[:],
                                 mybir.ActivationFunctionType.Relu,
                                 scale=1.0 - a_slope)
            nc.vector.scalar_tensor_tensor(
                sg[:], ps[:], a_slope, sg[:],
                op0=mybir.AluOpType.mult, op1=mybir.AluOpType.add)
            sg4 = sg.rearrange("p (b c d) -> p b c d", b=2, c=16, d=16)
            for aj in range(4):
                a = a0b * 4 + aj
                for s in range(4):
                    jlo, jcnt, dlo = SMAP[s]
                    dst = xn4[32 * s:32 * s + 32, a, b0:b0 + 2, 1:17,
                              jlo:jlo + jcnt]
                    src = sg4[32 * aj:32 * aj + 32, :, :,
                              dlo:dlo + 2 * jcnt - 1:2]
                    if s in (0, 2):
                        nc.scalar.copy(dst, src)
                    else:
                        nc.vector.tensor_copy(dst, src)

    # ================= layers 1..4 =================
    for l in range(1, 5):
        xc4 = xview(bufs[(l + 1) % 2])
        xn4 = xview(bufs[l % 2])
        wt = wmt[l - 1]
        a_slope = slopes[l]
        for a in range(la):
            for half in range(2):
                # one PSUM bank per col-packed chunk; interleave the two
                # chunks' matmuls so their PE streams overlap (different
                # column groups of the array)
                pst = [pp.tile([128, 512], F32, tag="ps", name=f"psm{ci}") for ci in range(2)]
                mms = []
                for gi, (da, db, dc) in enumerate(G_MID):
                    if not (0 <= a + da < la):
                        continue
                    mms.append((gi, da, db, dc))
                nmm = len(mms)
                prev_mm = None
                for idx, (gi, da, db, dc) in enumerate(mms):
                    for ci in range(2):
                        b0 = half * 8 + ci * 4
                        rb = ci * 64
                        blo = max(b0, -db)
                        bhi = min(b0 + 4, 16 - db)
                        rhs = xc4[:, a + da, blo + db:bhi + db,
                                  dc + 1:dc + 17, :]
                        out = pst[ci][rb:rb + 64,
                                      (blo - b0) * 128:(bhi - b0) * 128]
                        mm = nc.tensor.matmul(
                            out, wt[:, gi * 64:gi * 64 + 64], rhs,
                            start=(idx == 0), stop=(idx == nmm - 1),
                            tile_position=(0, rb))
                        # keep A/B col-group streams interleaved on the PE
                        # queue so they overlap on distinct array columns
                        if prev_mm is not None:
                            tile.add_dep_helper(mm.ins, prev_mm.ins, sync=False,
                                                reason="colpack order")
                        prev_mm = mm
                for ci in range(2):
                    b0 = half * 8 + ci * 4
                    rb = ci * 64
                    # t=0 -> s'=1 direct ; t=1 -> s'=2 direct
                    # prelu(x) = (1-a)*relu(x) + a*x in two single-psum ops
                    for t, sp in ((0, 1), (1, 2)):
                        dst = xn4[32 * sp:32 * sp + 32, a, b0:b0 + 4,
                                  1:17, 0:8]
                        src = pst[ci][rb + 32 * t:rb + 32 * t + 32, :]
                        nc.scalar.activation(
                            dst, src, mybir.ActivationFunctionType.Relu,
                            scale=1.0 - a_slope)
                        nc.vector.scalar_tensor_tensor(
                            dst, src, a_slope, dst,
                            op0=mybir.AluOpType.mult,
                            op1=mybir.AluOpType.add)
                    # j-shift copies: s'=3 <- s'=1 (j+1) ; s'=0 <- s'=2 (j-1)
                    # split across ACT/DVE so neither eviction engine stalls
                    # the PE stream
                    nc.scalar.copy(
                        xn4[96:128, a, b0:b0 + 4, 1:17, 0:7],
                        xn4[32:64, a, b0:b0 + 4, 1:17, 1:8])
                    nc.vector.tensor_copy(
                        xn4[0:32, a, b0:b0 + 4, 1:17, 1:8],
                        xn4[64:96, a, b0:b0 + 4, 1:17, 0:7])

    # ================= layer 5 =================
    xc4 = xview(bufs[0])
    for a in range(la):
        pst = [pp.tile([128, 512], F32, tag="ps", name=f"ps5{q}") for q in range(4)]
        mms = []
        for gi, (da, db, dc) in enumerate(G_MID):
            if not (0 <= a + da < la):
                continue
            mms.append((gi, da, db, dc))
        nmm = len(mms)
        prev_mm = None
        for idx, (gi, da, db, dc) in enumerate(mms):
            for q in range(4):
                b0 = q * 4
                blo = max(b0, -db)
                bhi = min(b0 + 4, 16 - db)
                rhs = xc4[:, a + da, blo + db:bhi + db, dc + 1:dc + 17, :]
                out = pst[q][32 * q:32 * q + 4,
                             (blo - b0) * 128:(bhi - b0) * 128]
                mm = nc.tensor.matmul(out, w5t[:, gi * 4:gi * 4 + 4], rhs,
                                      start=(idx == 0), stop=(idx == nmm - 1),
                                      tile_position=(0, 32 * q))
                if prev_mm is not None:
                    tile.add_dep_helper(mm.ins, prev_mm.ins, sync=False,
                                        reason="colpack order")
                prev_mm = mm
        for q in range(4):
            st = stg.tile([4, 512], F32, tag="stg")
            nc.vector.tensor_copy(st[:], pst[q][32 * q:32 * q + 4, :])
            cb = a * 2048 + q * 512
            nc.sync.dma_start(out_d[:, cb:cb + 512], st[:])


_CACHE = {}
LAST_RESULT = None


def _build(la, slopes):
    key = (la, tuple(slopes))
    if key not in _CACHE:
        nc = bacc.Bacc("TRN2")
        with tile.TileContext(nc) as tc:
            _conv_kernel(tc, la, slopes)
        nc.compile()
        _CACHE[key] = nc
    return _CACHE[key]


def kernel(x, k0, k1, k2, k3, k4, k5, slopes):
    x = np.asarray(x, np.float32)
    n, _, la = x.shape[:3]
    slopes_f = [float(s) for s in np.asarray(slopes, np.float32)]
    ws = _pack_weights((k0, k1, k2, k3, k4, k5), la)
    nc = _build(la, slopes_f)

    in_maps = []
    for i in range(n):
        m = {"xin": np.ascontiguousarray(x[i].reshape(2 * la, 4096)),
             "w0": ws[0], "w5": ws[5]}
        for l in (1, 2, 3, 4):
            m[f"w{l}"] = ws[l]
        in_maps.append(m)

    res = bass_utils.run_bass_kernel_spmd(nc, in_maps,
                                          core_ids=list(range(n)))
    global LAST_RESULT
    LAST_RESULT = res
    outs = []
    for i in range(n):
        od = res.results[i]["out_dec"].reshape(2, 2, la, 16, 16, 8)
        # [o, t, a, b, c, j] -> [o, a, b, c, j, t] -> d = 2j + t
        o = np.transpose(od, (0, 2, 3, 4, 5, 1)).reshape(2, la, 16, 16, 16)
        outs.append(o)
    return np.stack(outs).astype(np.float32)



# revision 2
# speedup vs baseline: 1.0371x; 1.0371x over previous
"""Trainium2 Bass kernel for a 6-layer 4D CNN (3^4 SAME convs + PReLU) — v2.

Problem: x (8, 2, 16,16,16,16) -> 6 conv layers, channels 2->32->32->32->32
->32->2, PReLU (scalar slope) after the first five convs.  Data-parallel over
batch N=8 across 8 cores.

v2 changes vs the v1 baseline (1824 us modeled):
  * Mid layers (1..4) use full M=128 matmuls: M = (2 a-outs x 2 d-outs x 32
    ch).  Each 2-wide a-output group accumulates streams from its <=4 source
    a-columns; the stationary for a source is a contiguous 128-col slice of a
    per-(db,dc) bank laid out [W(da=+1) | W(da=0) | W(da=-1)], so the
    (da_for_slot0, da_for_slot1) pair needed by each source pattern is one
    affine AP.  End sources use M=64 half matmuls (psum row offset).
    30 streams per (db,dc) pair per layer vs 46 at M=64: 812k -> 530k PE cyc.
  * Layer 5 uses an a-banded stationary: M = 64 = (16 a x 2 out-ch x 2
    d-outs); each source a streams once per (db,dc) into a +-1 band of psum
    rows (psum partition offset).  One [64,512] psum tile per b-chunk
    accumulates all 144 matmuls (zero-stationary matmul opens the group).
    812k/4-packed -> 283k PE cycles.
  * L0 input (d-shift replicated, padded, fp16) is packed on the host and
    DMA'd directly; the fp32 staging buffer and on-chip conversion are gone.
  * d-axis decimated activation layout X'[32*s + ch, (a, b, c_pad, j)] with
    d = 2*j + s - 1 is unchanged from v1, as are L0's a-partition scheme,
    PReLU eviction (ACT relu-scale + DVE scalar_tensor_tensor), and the
    j-shift copies that restore the redundant s=0/3 blocks.
"""

import sys

import numpy as np

for _p in ("/opt/trn_rl_repo", "/root/.axon_site/_ro/trn_rl_repo"):
    if _p not in sys.path:
        sys.path.append(_p)

import concourse.bass as bass  # noqa: E402
import concourse.mybir as mybir  # noqa: E402
import concourse.tile as tile  # noqa: E402
from concourse import bacc, bass_utils  # noqa: E402
from concourse._compat import with_exitstack  # noqa: E402

F32 = mybir.dt.float32
F16 = mybir.dt.float16

LB = 16
CP = 18   # padded c axis (c in -1..16)
DP = 18   # padded d axis in x_pad
J = 8     # d//2

# tap order: db=0 first so the first matmul of every psum group covers the
# full b-window (start=True zeroes the whole tile)
G9 = [(db, dc) for db in (0, -1, 1) for dc in (0, -1, 1)]
G_L0 = [(db, dc) for db in (0, -1, 1) for dc in (0, -1, 1)]


def _pack_weights(ks, la):
    """Host-side packing of conv kernels into stationary matrices (fp16)."""
    na4 = la // 4
    k0, k1, k2, k3, k4, k5 = [np.asarray(k, np.float32) for k in ks]

    # L0: W0[32*rg + i*la + a_in, (g*na4 + a0b)*128 + a_j*32 + o]
    # K = 96 = 3 d-shift blocks (rg -> dd = rg - 1) x (2 ch x la a_in, banded)
    w0 = np.zeros((128, len(G_L0) * na4 * 128), np.float32)
    for gi, (db, dc) in enumerate(G_L0):
        for a0b in range(na4):
            cb = (gi * na4 + a0b) * 128
            for rg in range(3):
                for aj in range(4):
                    for da in (-1, 0, 1):
                        ain = a0b * 4 + aj + da
                        if not (0 <= ain < la):
                            continue
                        for i in range(2):
                            w0[32 * rg + i * la + ain,
                               cb + aj * 32:cb + aj * 32 + 32] = \
                                k0[:, i, da + 1, db + 1, dc + 1, rg]

    # mid layers: per (db,dc) bank of three da blocks ordered (+1, 0, -1);
    # block col = t*32 + o; W(da)[32*s + i, t*32 + o] = k[o,i,da,db,dc,s-t]
    def pack_mid(k):
        w = np.zeros((128, 9 * 3 * 64), np.float32)
        for gi, (db, dc) in enumerate(G9):
            for bi, da in enumerate((1, 0, -1)):
                cb = (gi * 3 + bi) * 64
                for s in range(4):
                    for t in range(2):
                        if 0 <= s - t <= 2:
                            w[32 * s:32 * s + 32,
                              cb + t * 32:cb + t * 32 + 32] = \
                                k[:, :, da + 1, db + 1, dc + 1, s - t].T
        return w

    # L5 banded, full-M stationaries (psum row offsets must be 32-aligned):
    # per (gi, src) a [128, 64] matrix, col = 4*a' + 2*o + t nonzero only for
    # a' in the +-1 band of src (da = src - a').
    w5 = np.zeros((128, 9 * la * 64), np.float32)
    for gi, (db, dc) in enumerate(G9):
        for src in range(la):
            cb = (gi * la + src) * 64
            for ap_ in (src - 1, src, src + 1):
                if not (0 <= ap_ < la):
                    continue
                da = src - ap_
                for s in range(4):
                    for t in range(2):
                        if 0 <= s - t <= 2:
                            for o in range(2):
                                w5[32 * s:32 * s + 32,
                                   cb + 4 * ap_ + 2 * o + t] = \
                                    k5[o, :, da + 1, db + 1, dc + 1, s - t]

    return ([w0.astype(np.float16)] +
            [pack_mid(k).astype(np.float16) for k in (k1, k2, k3, k4)] +
            [w5.astype(np.float16)])


def _pack_x(x1, la):
    """One sample (2, la, 16, 16, 16) -> padded fp16 [128, 16*18*18].

    partition = 32*rg + i*la + a holds x[i, a] shifted in d by dd = rg - 1;
    b unpadded, c padded to 18 (data at 1..16), d padded to 18.
    """
    xp = np.zeros((128, LB, CP, DP), np.float16)
    x1 = np.asarray(x1, np.float16)
    for rg in range(3):
        for i in range(2):
            xp[32 * rg + i * la:32 * rg + i * la + la, :, 1:17,
               2 - rg:18 - rg] = x1[i]
    return np.ascontiguousarray(xp.reshape(128, LB * CP * DP))


@with_exitstack
def _conv_kernel(ctx, tc, la, slopes):
    """Emit the full 6-layer conv program. slopes: python floats len 5."""
    nc = tc.nc
    na4 = la // 4
    xcols = la * LB * CP * J

    xpad_d = nc.dram_tensor("xpad", [128, LB * CP * DP], F16,
                            kind="ExternalInput")
    w0_d = nc.dram_tensor("w0", [128, len(G_L0) * na4 * 128],
                          F16, kind="ExternalInput")
    wmid_d = [nc.dram_tensor(f"w{l}", [128, 9 * 3 * 64], F16,
                             kind="ExternalInput") for l in (1, 2, 3, 4)]
    w5_d = nc.dram_tensor("w5", [128, 9 * la * 64], F16, kind="ExternalInput")
    out_d = nc.dram_tensor("out_dec", [64, la * 128], F32,
                           kind="ExternalOutput")

    const = ctx.enter_context(tc.tile_pool(name="const", bufs=1))
    pp = ctx.enter_context(tc.tile_pool(name="ps", bufs=8, space="PSUM"))
    stg = ctx.enter_context(tc.tile_pool(name="stg", bufs=4))

    # ---- load weights + padded input ----
    xpad = const.tile([128, LB * CP * DP], F16)
    nc.sync.dma_start(xpad[:], xpad_d[:])
    w0t = const.tile([128, w0_d.shape[1]], F16)
    nc.sync.dma_start(w0t[:], w0_d[:])
    wmt = []
    for wd in wmid_d:
        t = const.tile([128, 9 * 3 * 64], F16, tag=wd.name)
        nc.sync.dma_start(t[:], wd[:])
        wmt.append(t)
    w5t = const.tile([128, 9 * la * 64], F16)
    nc.sync.dma_start(w5t[:], w5_d[:])

    xp4 = xpad.rearrange("p (b c d) -> p b c d", b=LB, c=CP, d=DP)

    # ---- X' ping-pong buffers ----
    xa = const.tile([128, xcols], F16)
    xb = const.tile([128, xcols], F16)
    nc.gpsimd.memset(xa[:], 0.0)
    nc.gpsimd.memset(xb[:], 0.0)
    bufs = [xa, xb]

    def xview(t):
        return t.rearrange("p (a b c j) -> p a b c j", a=la, b=LB, c=CP, j=J)

    # scatter map: s -> (j_lo, j_cnt, d_lo)  [d = 2j + s - 1]
    SMAP = {0: (1, 7, 1), 1: (0, 8, 0), 2: (0, 8, 1), 3: (0, 7, 2)}

    # ================= layer 0 =================
    xn4 = xview(bufs[0])
    a_slope = slopes[0]
    for a0b in range(na4):
        for bc in range(8):          # b-pair chunks
            b0 = bc * 2
            ps = pp.tile([128, 512], F32, tag="ps")
            p4 = ps.rearrange("p (b c d) -> p b c d", b=2, c=16, d=16)
            for gi, (db, dc) in enumerate(G_L0):
                blo = max(b0, -db)
                bhi = min(b0 + 2, 16 - db)
                cb = (gi * na4 + a0b) * 128
                # K = 96: three d-shift blocks; rhs d-slice 1:17 uniform
                rhs = xp4[0:96, blo + db:bhi + db, dc + 1:dc + 17, 1:17]
                out = p4[:, blo - b0:bhi - b0, :, :]
                nc.tensor.matmul(out, w0t[0:96, cb:cb + 128], rhs,
                                 start=(gi == 0),
                                 stop=(gi == len(G_L0) - 1))
            # prelu the whole chunk into fp16 staging, then scatter
            sg = stg.tile([128, 512], F16, tag="l0st")
            nc.scalar.activation(sg[:], ps[:],
                                 mybir.ActivationFunctionType.Relu,
                                 scale=1.0 - a_slope)
            nc.vector.scalar_tensor_tensor(
                sg[:], ps[:], a_slope, sg[:],
                op0=mybir.AluOpType.mult, op1=mybir.AluOpType.add)
            sg4 = sg.rearrange("p (b c d) -> p b c d", b=2, c=16, d=16)
            for aj in range(4):
                a = a0b * 4 + aj
                for s in range(4):
                    jlo, jcnt, dlo = SMAP[s]
                    dst = xn4[32 * s:32 * s + 32, a, b0:b0 + 2, 1:17,
                              jlo:jlo + jcnt]
                    src = sg4[32 * aj:32 * aj + 32, :, :,
                              dlo:dlo + 2 * jcnt - 1:2]
                    if s in (0, 2):
                        nc.scalar.copy(dst, src)
                    else:
                        nc.vector.tensor_copy(dst, src)

    # ================= layers 1..4 =================
    # M = 128 = (2 a-outs x 2 t x 32 ch); per (db,dc) the stationary for
    # source pattern p (src = a0 - 1 + p) is the contiguous da-block pair
    # (2-p, 3-p) of the bank [W(+1) | W(0) | W(-1)]; p=0/3 are M=64 halves.
    for l in range(1, 5):
        xc4 = xview(bufs[(l + 1) % 2])
        xn4 = xview(bufs[l % 2])
        wt = wmt[l - 1]
        a_slope = slopes[l]
        for g in range(8):
            a0 = 2 * g
            for chunk in range(4):
                b0 = 4 * chunk
                ps = pp.tile([128, 512], F32, tag="ps")
                mms = []
                for gi, (db, dc) in enumerate(G9):
                    for p in (1, 2, 0, 3):   # full-M patterns first
                        src = a0 - 1 + p
                        if not (0 <= src < la):
                            continue
                        mms.append((gi, db, dc, p, src))
                nmm = len(mms)
                for idx, (gi, db, dc, p, src) in enumerate(mms):
                    blo = max(b0, -db)
                    bhi = min(b0 + 4, 16 - db)
                    rhs = xc4[:, src, blo + db:bhi + db, dc + 1:dc + 17, :]
                    cw = (blo - b0) * 128, (bhi - b0) * 128
                    if p in (1, 2):
                        lhsT = wt[:, (gi * 3 + 2 - p) * 64:
                                  (gi * 3 + 4 - p) * 64]
                        outv = ps[:, cw[0]:cw[1]]
                    elif p == 0:   # only slot q=0 (out a0), da = -1
                        lhsT = wt[:, (gi * 3 + 2) * 64:(gi * 3 + 3) * 64]
                        outv = ps[0:64, cw[0]:cw[1]]
                    else:          # p == 3: only slot q=1 (out a0+1), da = +1
                        lhsT = wt[:, (gi * 3 + 0) * 64:(gi * 3 + 1) * 64]
                        outv = ps[64:128, cw[0]:cw[1]]
                    nc.tensor.matmul(outv, lhsT, rhs,
                                     start=(idx == 0), stop=(idx == nmm - 1))
                # prelu evictions: psum row block (64q + 32t) -> s' = t + 1
                for q in (0, 1):
                    ap = a0 + q
                    for t in (0, 1):
                        dst = xn4[32 * (t + 1):32 * (t + 2), ap,
                                  b0:b0 + 4, 1:17, 0:8]
                        srcv = ps[64 * q + 32 * t:64 * q + 32 * t + 32, :]
                        nc.scalar.activation(
                            dst, srcv, mybir.ActivationFunctionType.Relu,
                            scale=1.0 - a_slope)
                        nc.vector.scalar_tensor_tensor(
                            dst, srcv, a_slope, dst,
                            op0=mybir.AluOpType.mult,
                            op1=mybir.AluOpType.add)
                    # j-shift copies: s'=3 <- s'=1 (j+1) ; s'=0 <- s'=2 (j-1)
                    nc.scalar.copy(
                        xn4[96:128, ap, b0:b0 + 4, 1:17, 0:7],
                        xn4[32:64, ap, b0:b0 + 4, 1:17, 1:8])
                    nc.vector.tensor_copy(
                        xn4[0:32, ap, b0:b0 + 4, 1:17, 1:8],
                        xn4[64:96, ap, b0:b0 + 4, 1:17, 0:7])

    # ================= layer 5 (a-banded, M=64 = 16a x 2o x 2t) =================
    xc4 = xview(bufs[0])
    for chunk in range(4):
        b0 = 4 * chunk
        psf = pp.tile([128, 512], F32, tag="ps")
        ps = psf[0:64, :]
        mms = []
        for gi, (db, dc) in enumerate(G9):
            for src in range(la):
                mms.append((gi, db, dc, src))
        nmm = len(mms)
        for idx, (gi, db, dc, src) in enumerate(mms):
            blo = max(b0, -db)
            bhi = min(b0 + 4, 16 - db)
            rhs = xc4[:, src, blo + db:bhi + db, dc + 1:dc + 17, :]
            lhsT = w5t[:, (gi * la + src) * 64:(gi * la + src) * 64 + 64]
            outv = ps[:, (blo - b0) * 128:(bhi - b0) * 128]
            nc.tensor.matmul(outv, lhsT, rhs,
                             start=(idx == 0), stop=(idx == nmm - 1))
        st = stg.tile([64, 512], F32, tag="stg")
        nc.vector.tensor_copy(st[:], ps[:])
        nc.sync.dma_start(out_d[:, chunk * 512:chunk * 512 + 512], st[:])


_CACHE = {}
LAST_RESULT = None


def _build(la, slopes):
    key = (la, tuple(slopes))
    if key not in _CACHE:
        nc = bacc.Bacc("TRN2")
        with tile.TileContext(nc) as tc:
            _conv_kernel(tc, la, slopes)
        nc.compile()
        _CACHE[key] = nc
    return _CACHE[key]


def kernel(x, k0, k1, k2, k3, k4, k5, slopes):
    x = np.asarray(x, np.float32)
    n, _, la = x.shape[:3]
    slopes_f = [float(s) for s in np.asarray(slopes, np.float32)]
    ws = _pack_weights((k0, k1, k2, k3, k4, k5), la)
    nc = _build(la, slopes_f)

    in_maps = []
    for i in range(n):
        m = {"xpad": _pack_x(x[i], la), "w0": ws[0], "w5": ws[5]}
        for l in (1, 2, 3, 4):
            m[f"w{l}"] = ws[l]
        in_maps.append(m)

    res = bass_utils.run_bass_kernel_spmd(nc, in_maps,
                                          core_ids=list(range(n)))
    global LAST_RESULT
    LAST_RESULT = res
    outs = []
    for i in range(n):
        od = res.results[i]["out_dec"].reshape(la, 2, 2, 16, 16, 8)
        # rows (a, o, t) x cols (b, c, j) -> [o, a, b, c, d = 2j + t]
        o = np.transpose(od, (1, 0, 3, 4, 5, 2)).reshape(2, la, 16, 16, 16)
        outs.append(o)
    return np.stack(outs).astype(np.float32)


# revision 3
# speedup vs baseline: 1.0492x; 1.0117x over previous
"""Trainium2 Bass kernel for a 6-layer 4D CNN (3^4 SAME convs + PReLU) — v2.

Problem: x (8, 2, 16,16,16,16) -> 6 conv layers, channels 2->32->32->32->32
->32->2, PReLU (scalar slope) after the first five convs.  Data-parallel over
batch N=8 across 8 cores.

v2 changes vs the v1 baseline (1824 us modeled):
  * Mid layers (1..4) use full M=128 matmuls: M = (2 a-outs x 2 d-outs x 32
    ch).  Each 2-wide a-output group accumulates streams from its <=4 source
    a-columns; the stationary for a source is a contiguous 128-col slice of a
    per-(db,dc) bank laid out [W(da=+1) | W(da=0) | W(da=-1)], so the
    (da_for_slot0, da_for_slot1) pair needed by each source pattern is one
    affine AP.  End sources use M=64 half matmuls (psum row offset).
    30 streams per (db,dc) pair per layer vs 46 at M=64: 812k -> 530k PE cyc.
  * Layer 5 uses an a-banded stationary: M = 64 = (16 a x 2 out-ch x 2
    d-outs); each source a streams once per (db,dc) into a +-1 band of psum
    rows (psum partition offset).  One [64,512] psum tile per b-chunk
    accumulates all 144 matmuls (zero-stationary matmul opens the group).
    812k/4-packed -> 283k PE cycles.
  * L0 input (d-shift replicated, padded, fp16) is packed on the host and
    DMA'd directly; the fp32 staging buffer and on-chip conversion are gone.
  * d-axis decimated activation layout X'[32*s + ch, (a, b, c_pad, j)] with
    d = 2*j + s - 1 is unchanged from v1, as are L0's a-partition scheme,
    PReLU eviction (ACT relu-scale + DVE scalar_tensor_tensor), and the
    j-shift copies that restore the redundant s=0/3 blocks.
"""

import sys

import numpy as np

for _p in ("/opt/trn_rl_repo", "/root/.axon_site/_ro/trn_rl_repo"):
    if _p not in sys.path:
        sys.path.append(_p)

import concourse.bass as bass  # noqa: E402
import concourse.mybir as mybir  # noqa: E402
import concourse.tile as tile  # noqa: E402
from concourse import bacc, bass_utils  # noqa: E402
from concourse._compat import with_exitstack  # noqa: E402

F32 = mybir.dt.float32
F16 = mybir.dt.float16

LB = 16
CP = 18   # padded c axis (c in -1..16)
DP = 18   # padded d axis in x_pad
J = 8     # d//2

# tap order: db=0 first so the first matmul of every psum group covers the
# full b-window (start=True zeroes the whole tile)
G9 = [(db, dc) for db in (0, -1, 1) for dc in (0, -1, 1)]
G_L0 = [(db, dc) for db in (0, -1, 1) for dc in (0, -1, 1)]


def _pack_weights(ks, la):
    """Host-side packing of conv kernels into stationary matrices (fp16)."""
    na4 = la // 4
    k0, k1, k2, k3, k4, k5 = [np.asarray(k, np.float32) for k in ks]

    # L0: W0[32*rg + i*la + a_in, (a0b*9 + g)*128 + a_j*32 + o]  (a0b-major
    # so the DMA can be sliced per a0b block)
    # K = 96 = 3 d-shift blocks (rg -> dd = rg - 1) x (2 ch x la a_in, banded)
    w0 = np.zeros((128, len(G_L0) * na4 * 128), np.float32)
    for gi, (db, dc) in enumerate(G_L0):
        for a0b in range(na4):
            cb = (a0b * len(G_L0) + gi) * 128
            for rg in range(3):
                for aj in range(4):
                    for da in (-1, 0, 1):
                        ain = a0b * 4 + aj + da
                        if not (0 <= ain < la):
                            continue
                        for i in range(2):
                            w0[32 * rg + i * la + ain,
                               cb + aj * 32:cb + aj * 32 + 32] = \
                                k0[:, i, da + 1, db + 1, dc + 1, rg]

    # mid layers: per (db,dc) bank of three da blocks ordered (+1, 0, -1);
    # block col = t*32 + o; W(da)[32*s + i, t*32 + o] = k[o,i,da,db,dc,s-t]
    def pack_mid(k):
        w = np.zeros((128, 9 * 3 * 64), np.float32)
        for gi, (db, dc) in enumerate(G9):
            for bi, da in enumerate((1, 0, -1)):
                cb = (gi * 3 + bi) * 64
                for s in range(4):
                    for t in range(2):
                        if 0 <= s - t <= 2:
                            w[32 * s:32 * s + 32,
                              cb + t * 32:cb + t * 32 + 32] = \
                                k[:, :, da + 1, db + 1, dc + 1, s - t].T
        return w

    # L5 banded, full-M stationaries (psum row offsets must be 32-aligned):
    # per (gi, src) a [128, 64] matrix, col = 4*a' + 2*o + t nonzero only for
    # a' in the +-1 band of src (da = src - a').
    w5 = np.zeros((128, 9 * la * 64), np.float32)
    for gi, (db, dc) in enumerate(G9):
        for src in range(la):
            cb = (gi * la + src) * 64
            for ap_ in (src - 1, src, src + 1):
                if not (0 <= ap_ < la):
                    continue
                da = src - ap_
                for s in range(4):
                    for t in range(2):
                        if 0 <= s - t <= 2:
                            for o in range(2):
                                w5[32 * s:32 * s + 32,
                                   cb + 4 * ap_ + 2 * o + t] = \
                                    k5[o, :, da + 1, db + 1, dc + 1, s - t]

    return ([w0.astype(np.float16)] +
            [pack_mid(k).astype(np.float16) for k in (k1, k2, k3, k4)] +
            [w5.astype(np.float16)])


def _pack_x(x1, la):
    """One sample (2, la, 16, 16, 16) -> padded fp16 [128, 16*18*18].

    partition = 32*rg + i*la + a holds x[i, a] shifted in d by dd = rg - 1;
    b unpadded, c padded to 18 (data at 1..16), d padded to 18.
    """
    xp = np.zeros((128, LB, CP, DP), np.float16)
    x1 = np.asarray(x1, np.float16)
    for rg in range(3):
        for i in range(2):
            xp[32 * rg + i * la:32 * rg + i * la + la, :, 1:17,
               2 - rg:18 - rg] = x1[i]
    return np.ascontiguousarray(xp.reshape(128, LB * CP * DP))


@with_exitstack
def _conv_kernel(ctx, tc, la, slopes):
    """Emit the full 6-layer conv program. slopes: python floats len 5."""
    nc = tc.nc
    na4 = la // 4
    xcols = la * LB * CP * J

    xpad_d = nc.dram_tensor("xpad", [128, LB * CP * DP], F16,
                            kind="ExternalInput")
    w0_d = nc.dram_tensor("w0", [128, len(G_L0) * na4 * 128],
                          F16, kind="ExternalInput")
    wmid_d = [nc.dram_tensor(f"w{l}", [128, 9 * 3 * 64], F16,
                             kind="ExternalInput") for l in (1, 2, 3, 4)]
    w5_d = nc.dram_tensor("w5", [128, 9 * la * 64], F16, kind="ExternalInput")
    out_d = nc.dram_tensor("out_dec", [64, la * 128], F32,
                           kind="ExternalOutput")

    const = ctx.enter_context(tc.tile_pool(name="const", bufs=1))
    pp = ctx.enter_context(tc.tile_pool(name="ps", bufs=8, space="PSUM"))
    stg = ctx.enter_context(tc.tile_pool(name="stg", bufs=4))

    # ---- load weights + padded input ----
    # xpad and w0 are DMA'd in b-/a0b-slices so layer 0's first chunks are
    # gated by ~0.6MB of DMA instead of the full 2.5MB.
    xpad = const.tile([128, LB * CP * DP], F16)
    xpv = xpad.rearrange("p (b cd) -> p b cd", b=LB, cd=CP * DP)
    xdv = xpad_d.rearrange("p (b cd) -> p b cd", b=LB, cd=CP * DP)
    for b0 in range(0, LB, 4):
        nc.sync.dma_start(xpv[:, b0:b0 + 4, :], xdv[:, b0:b0 + 4, :])
    w0t = const.tile([128, w0_d.shape[1]], F16)
    for a0b in range(na4):
        cb = a0b * 9 * 128
        nc.sync.dma_start(w0t[:, cb:cb + 9 * 128], w0_d[:, cb:cb + 9 * 128])
    wmt = []
    for wd in wmid_d:
        t = const.tile([128, 9 * 3 * 64], F16, tag=wd.name)
        nc.sync.dma_start(t[:], wd[:])
        wmt.append(t)
    w5t = const.tile([128, 9 * la * 64], F16)
    nc.sync.dma_start(w5t[:], w5_d[:])

    xp4 = xpad.rearrange("p (b c d) -> p b c d", b=LB, c=CP, d=DP)

    # ---- X' ping-pong buffers ----
    xa = const.tile([128, xcols], F16)
    xb = const.tile([128, xcols], F16)
    bufs = [xa, xb]

    def xview(t):
        return t.rearrange("p (a b c j) -> p a b c j", a=la, b=LB, c=CP, j=J)

    # Only the padding regions need zeroing (evictions write everything else
    # before it is read): c = 0 / 17 columns, plus the j-edge zeros of the
    # redundant d blocks (s=0 @ j=0 -> d=-1, s=3 @ j=7 -> d=16).
    for t in bufs:
        v = xview(t)
        nc.gpsimd.memset(v[:, :, :, 0:1, :], 0.0)
        nc.gpsimd.memset(v[:, :, :, 17:18, :], 0.0)
        nc.gpsimd.memset(v[0:32, :, :, 1:17, 0:1], 0.0)
        nc.gpsimd.memset(v[96:128, :, :, 1:17, 7:8], 0.0)

    # scatter map: s -> (j_lo, j_cnt, d_lo)  [d = 2j + s - 1]
    SMAP = {0: (1, 7, 1), 1: (0, 8, 0), 2: (0, 8, 1), 3: (0, 7, 2)}

    # ================= layer 0 =================
    xn4 = xview(bufs[0])
    a_slope = slopes[0]
    for a0b in range(na4):
        for bc in range(8):          # b-pair chunks
            b0 = bc * 2
            ps = pp.tile([128, 512], F32, tag="ps")
            p4 = ps.rearrange("p (b c d) -> p b c d", b=2, c=16, d=16)
            for gi, (db, dc) in enumerate(G_L0):
                blo = max(b0, -db)
                bhi = min(b0 + 2, 16 - db)
                cb = (a0b * len(G_L0) + gi) * 128
                # K = 96: three d-shift blocks; rhs d-slice 1:17 uniform
                rhs = xp4[0:96, blo + db:bhi + db, dc + 1:dc + 17, 1:17]
                out = p4[:, blo - b0:bhi - b0, :, :]
                nc.tensor.matmul(out, w0t[0:96, cb:cb + 128], rhs,
                                 start=(gi == 0),
                                 stop=(gi == len(G_L0) - 1))
            # prelu in ONE ACT op (Lrelu, alpha=slope), then scatter the
            # chunk across DVE / GPSIMD / ACT so the scatter keeps up with
            # the PE during the L0 phase.
            sg = stg.tile([128, 512], F16, tag="l0st")
            nc.scalar.activation(sg[:], ps[:],
                                 mybir.ActivationFunctionType.Lrelu,
                                 alpha=a_slope)
            sg4 = sg.rearrange("p (b c d) -> p b c d", b=2, c=16, d=16)
            eng = 0
            for aj in range(4):
                a = a0b * 4 + aj
                for s in range(4):
                    jlo, jcnt, dlo = SMAP[s]
                    dst = xn4[32 * s:32 * s + 32, a, b0:b0 + 2, 1:17,
                              jlo:jlo + jcnt]
                    src = sg4[32 * aj:32 * aj + 32, :, :,
                              dlo:dlo + 2 * jcnt - 1:2]
                    if eng in (0, 3):
                        nc.vector.tensor_copy(dst, src)
                    elif eng in (1, 4):
                        nc.gpsimd.tensor_copy(dst, src)
                    else:
                        nc.scalar.copy(dst, src)
                    eng = (eng + 1) % 5

    # ================= layers 1..4 =================
    # M = 128 = (2 a-outs x 2 t x 32 ch); per (db,dc) the stationary for
    # source pattern p (src = a0 - 1 + p) is the contiguous da-block pair
    # (2-p, 3-p) of the bank [W(+1) | W(0) | W(-1)]; p=0/3 are M=64 halves.
    for l in range(1, 5):
        xc4 = xview(bufs[(l + 1) % 2])
        xn4 = xview(bufs[l % 2])
        wt = wmt[l - 1]
        a_slope = slopes[l]
        for g in range(8):
            a0 = 2 * g
            for chunk in range(4):
                b0 = 4 * chunk
                ps = pp.tile([128, 512], F32, tag="ps")
                mms = []
                for gi, (db, dc) in enumerate(G9):
                    for p in (1, 2, 0, 3):   # full-M patterns first
                        src = a0 - 1 + p
                        if not (0 <= src < la):
                            continue
                        mms.append((gi, db, dc, p, src))
                nmm = len(mms)
                for idx, (gi, db, dc, p, src) in enumerate(mms):
                    blo = max(b0, -db)
                    bhi = min(b0 + 4, 16 - db)
                    rhs = xc4[:, src, blo + db:bhi + db, dc + 1:dc + 17, :]
                    cw = (blo - b0) * 128, (bhi - b0) * 128
                    if p in (1, 2):
                        lhsT = wt[:, (gi * 3 + 2 - p) * 64:
                                  (gi * 3 + 4 - p) * 64]
                        outv = ps[:, cw[0]:cw[1]]
                    elif p == 0:   # only slot q=0 (out a0), da = -1
                        lhsT = wt[:, (gi * 3 + 2) * 64:(gi * 3 + 3) * 64]
                        outv = ps[0:64, cw[0]:cw[1]]
                    else:          # p == 3: only slot q=1 (out a0+1), da = +1
                        lhsT = wt[:, (gi * 3 + 0) * 64:(gi * 3 + 1) * 64]
                        outv = ps[64:128, cw[0]:cw[1]]
                    nc.tensor.matmul(outv, lhsT, rhs,
                                     start=(idx == 0), stop=(idx == nmm - 1))
                # prelu evictions: psum row block (64q + 32t) -> s' = t + 1;
                # one Lrelu ACT op per block; j-shift copies on DVE/GPSIMD
                for q in (0, 1):
                    ap = a0 + q
                    for t in (0, 1):
                        dst = xn4[32 * (t + 1):32 * (t + 2), ap,
                                  b0:b0 + 4, 1:17, 0:8]
                        srcv = ps[64 * q + 32 * t:64 * q + 32 * t + 32, :]
                        nc.scalar.activation(
                            dst, srcv, mybir.ActivationFunctionType.Lrelu,
                            alpha=a_slope)
                    # j-shift copies: s'=3 <- s'=1 (j+1) ; s'=0 <- s'=2 (j-1)
                    nc.vector.tensor_copy(
                        xn4[96:128, ap, b0:b0 + 4, 1:17, 0:7],
                        xn4[32:64, ap, b0:b0 + 4, 1:17, 1:8])
                    nc.gpsimd.tensor_copy(
                        xn4[0:32, ap, b0:b0 + 4, 1:17, 1:8],
                        xn4[64:96, ap, b0:b0 + 4, 1:17, 0:7])

    # ================= layer 5 (a-banded, M=64 = 16a x 2o x 2t) =================
    xc4 = xview(bufs[0])
    for chunk in range(4):
        b0 = 4 * chunk
        psf = pp.tile([128, 512], F32, tag="ps")
        ps = psf[0:64, :]
        mms = []
        for gi, (db, dc) in enumerate(G9):
            for src in range(la):
                mms.append((gi, db, dc, src))
        nmm = len(mms)
        for idx, (gi, db, dc, src) in enumerate(mms):
            blo = max(b0, -db)
            bhi = min(b0 + 4, 16 - db)
            rhs = xc4[:, src, blo + db:bhi + db, dc + 1:dc + 17, :]
            lhsT = w5t[:, (gi * la + src) * 64:(gi * la + src) * 64 + 64]
            outv = ps[:, (blo - b0) * 128:(bhi - b0) * 128]
            nc.tensor.matmul(outv, lhsT, rhs,
                             start=(idx == 0), stop=(idx == nmm - 1))
        st = stg.tile([64, 512], F32, tag="stg")
        nc.vector.tensor_copy(st[:], ps[:])
        nc.sync.dma_start(out_d[:, chunk * 512:chunk * 512 + 512], st[:])


_CACHE = {}
LAST_RESULT = None


def _build(la, slopes):
    key = (la, tuple(slopes))
    if key not in _CACHE:
        nc = bacc.Bacc("TRN2")
        with tile.TileContext(nc) as tc:
            _conv_kernel(tc, la, slopes)
        nc.compile()
        _CACHE[key] = nc
    return _CACHE[key]


def kernel(x, k0, k1, k2, k3, k4, k5, slopes):
    x = np.asarray(x, np.float32)
    n, _, la = x.shape[:3]
    slopes_f = [float(s) for s in np.asarray(slopes, np.float32)]
    ws = _pack_weights((k0, k1, k2, k3, k4, k5), la)
    nc = _build(la, slopes_f)

    in_maps = []
    for i in range(n):
        m = {"xpad": _pack_x(x[i], la), "w0": ws[0], "w5": ws[5]}
        for l in (1, 2, 3, 4):
            m[f"w{l}"] = ws[l]
        in_maps.append(m)

    res = bass_utils.run_bass_kernel_spmd(nc, in_maps,
                                          core_ids=list(range(n)))
    global LAST_RESULT
    LAST_RESULT = res
    outs = []
    for i in range(n):
        od = res.results[i]["out_dec"].reshape(la, 2, 2, 16, 16, 8)
        # rows (a, o, t) x cols (b, c, j) -> [o, a, b, c, d = 2j + t]
        o = np.transpose(od, (1, 0, 3, 4, 5, 2)).reshape(2, la, 16, 16, 16)
        outs.append(o)
    return np.stack(outs).astype(np.float32)


# revision 4
# speedup vs baseline: 1.0505x; 1.0013x over previous
"""Trainium2 Bass kernel for a 6-layer 4D CNN (3^4 SAME convs + PReLU) — v2.

Problem: x (8, 2, 16,16,16,16) -> 6 conv layers, channels 2->32->32->32->32
->32->2, PReLU (scalar slope) after the first five convs.  Data-parallel over
batch N=8 across 8 cores.

v2 changes vs the v1 baseline (1824 us modeled):
  * Mid layers (1..4) use full M=128 matmuls: M = (2 a-outs x 2 d-outs x 32
    ch).  Each 2-wide a-output group accumulates streams from its <=4 source
    a-columns; the stationary for a source is a contiguous 128-col slice of a
    per-(db,dc) bank laid out [W(da=+1) | W(da=0) | W(da=-1)], so the
    (da_for_slot0, da_for_slot1) pair needed by each source pattern is one
    affine AP.  End sources use M=64 half matmuls (psum row offset).
    30 streams per (db,dc) pair per layer vs 46 at M=64: 812k -> 530k PE cyc.
  * Layer 5 uses an a-banded stationary: M = 64 = (16 a x 2 out-ch x 2
    d-outs); each source a streams once per (db,dc) into a +-1 band of psum
    rows (psum partition offset).  One [64,512] psum tile per b-chunk
    accumulates all 144 matmuls (zero-stationary matmul opens the group).
    812k/4-packed -> 283k PE cycles.
  * L0 input (d-shift replicated, padded, fp16) is packed on the host and
    DMA'd directly; the fp32 staging buffer and on-chip conversion are gone.
  * d-axis decimated activation layout X'[32*s + ch, (a, b, c_pad, j)] with
    d = 2*j + s - 1 is unchanged from v1, as are L0's a-partition scheme,
    PReLU eviction (ACT relu-scale + DVE scalar_tensor_tensor), and the
    j-shift copies that restore the redundant s=0/3 blocks.
"""

import sys

import numpy as np

for _p in ("/opt/trn_rl_repo", "/root/.axon_site/_ro/trn_rl_repo"):
    if _p not in sys.path:
        sys.path.append(_p)

import concourse.bass as bass  # noqa: E402
import concourse.mybir as mybir  # noqa: E402
import concourse.tile as tile  # noqa: E402
from concourse import bacc, bass_utils  # noqa: E402
from concourse._compat import with_exitstack  # noqa: E402

F32 = mybir.dt.float32
F16 = mybir.dt.float16

LB = 16
CP = 18   # padded c axis (c in -1..16)
DP = 18   # padded d axis in x_pad
J = 8     # d//2

# tap order: db=0 first so the first matmul of every psum group covers the
# full b-window (start=True zeroes the whole tile)
G9 = [(db, dc) for db in (0, -1, 1) for dc in (0, -1, 1)]
G_L0 = [(db, dc) for db in (0, -1, 1) for dc in (0, -1, 1)]


def _pack_weights(ks, la):
    """Host-side packing of conv kernels into stationary matrices (fp16)."""
    na4 = la // 4
    k0, k1, k2, k3, k4, k5 = [np.asarray(k, np.float32) for k in ks]

    # L0: W0[32*rg + i*la + a_in, (a0b*9 + g)*128 + a_j*32 + o]  (a0b-major
    # so the DMA can be sliced per a0b block)
    # K = 96 = 3 d-shift blocks (rg -> dd = rg - 1) x (2 ch x la a_in, banded)
    w0 = np.zeros((128, len(G_L0) * na4 * 128), np.float32)
    for gi, (db, dc) in enumerate(G_L0):
        for a0b in range(na4):
            cb = (a0b * len(G_L0) + gi) * 128
            for rg in range(3):
                for aj in range(4):
                    for da in (-1, 0, 1):
                        ain = a0b * 4 + aj + da
                        if not (0 <= ain < la):
                            continue
                        for i in range(2):
                            w0[32 * rg + i * la + ain,
                               cb + aj * 32:cb + aj * 32 + 32] = \
                                k0[:, i, da + 1, db + 1, dc + 1, rg]

    # mid layers: per (db,dc) bank of three da blocks ordered (+1, 0, -1);
    # block col = t*32 + o; W(da)[32*s + i, t*32 + o] = k[o,i,da,db,dc,s-t]
    def pack_mid(k):
        w = np.zeros((128, 9 * 3 * 64), np.float32)
        for gi, (db, dc) in enumerate(G9):
            for bi, da in enumerate((1, 0, -1)):
                cb = (gi * 3 + bi) * 64
                for s in range(4):
                    for t in range(2):
                        if 0 <= s - t <= 2:
                            w[32 * s:32 * s + 32,
                              cb + t * 32:cb + t * 32 + 32] = \
                                k[:, :, da + 1, db + 1, dc + 1, s - t].T
        return w

    # L5 banded, full-M stationaries (psum row offsets must be 32-aligned):
    # per (gi, src) a [128, 64] matrix, col = 4*a' + 2*o + t nonzero only for
    # a' in the +-1 band of src (da = src - a').
    w5 = np.zeros((128, 9 * la * 64), np.float32)
    for gi, (db, dc) in enumerate(G9):
        for src in range(la):
            cb = (gi * la + src) * 64
            for ap_ in (src - 1, src, src + 1):
                if not (0 <= ap_ < la):
                    continue
                da = src - ap_
                for s in range(4):
                    for t in range(2):
                        if 0 <= s - t <= 2:
                            for o in range(2):
                                w5[32 * s:32 * s + 32,
                                   cb + 4 * ap_ + 2 * o + t] = \
                                    k5[o, :, da + 1, db + 1, dc + 1, s - t]

    return ([w0.astype(np.float16)] +
            [pack_mid(k).astype(np.float16) for k in (k1, k2, k3, k4)] +
            [w5.astype(np.float16)])


def _pack_x(x1, la):
    """One sample (2, la, 16, 16, 16) -> padded fp16 [128, 16*18*18].

    partition = 32*rg + i*la + a holds x[i, a] shifted in d by dd = rg - 1;
    b unpadded, c padded to 18 (data at 1..16), d padded to 18.
    """
    xp = np.zeros((128, LB, CP, DP), np.float16)
    x1 = np.asarray(x1, np.float16)
    for rg in range(3):
        for i in range(2):
            xp[32 * rg + i * la:32 * rg + i * la + la, :, 1:17,
               2 - rg:18 - rg] = x1[i]
    return np.ascontiguousarray(xp.reshape(128, LB * CP * DP))


@with_exitstack
def _conv_kernel(ctx, tc, la, slopes):
    """Emit the full 6-layer conv program. slopes: python floats len 5."""
    nc = tc.nc
    na4 = la // 4
    xcols = la * LB * CP * J

    xpad_d = nc.dram_tensor("xpad", [128, LB * CP * DP], F16,
                            kind="ExternalInput")
    w0_d = nc.dram_tensor("w0", [128, len(G_L0) * na4 * 128],
                          F16, kind="ExternalInput")
    wmid_d = [nc.dram_tensor(f"w{l}", [128, 9 * 3 * 64], F16,
                             kind="ExternalInput") for l in (1, 2, 3, 4)]
    w5_d = nc.dram_tensor("w5", [128, 9 * la * 64], F16, kind="ExternalInput")
    out_d = nc.dram_tensor("out_dec", [64, la * 128], F32,
                           kind="ExternalOutput")

    const = ctx.enter_context(tc.tile_pool(name="const", bufs=1))
    pp = ctx.enter_context(tc.tile_pool(name="ps", bufs=8, space="PSUM"))
    stg = ctx.enter_context(tc.tile_pool(name="stg", bufs=4))

    # ---- load weights + padded input ----
    # xpad and w0 are DMA'd in b-/a0b-slices so layer 0's first chunks are
    # gated by ~0.6MB of DMA instead of the full 2.5MB.
    xpad = const.tile([128, LB * CP * DP], F16)
    xpv = xpad.rearrange("p (b cd) -> p b cd", b=LB, cd=CP * DP)
    xdv = xpad_d.rearrange("p (b cd) -> p b cd", b=LB, cd=CP * DP)
    w0t = const.tile([128, w0_d.shape[1]], F16)
    # interleave xpad/w0 slices so the first L0 chunk is gated by only the
    # first xpad b-slice + first w0 block (~0.6MB), not the whole 2.5MB
    for a0b in range(na4):
        b0 = a0b * 4
        nc.sync.dma_start(xpv[:, b0:b0 + 4, :], xdv[:, b0:b0 + 4, :])
        cb = a0b * 9 * 128
        nc.sync.dma_start(w0t[:, cb:cb + 9 * 128], w0_d[:, cb:cb + 9 * 128])
    wmt = []
    for wd in wmid_d:
        t = const.tile([128, 9 * 3 * 64], F16, tag=wd.name)
        nc.sync.dma_start(t[:], wd[:])
        wmt.append(t)
    w5t = const.tile([128, 9 * la * 64], F16)
    nc.sync.dma_start(w5t[:], w5_d[:])

    xp4 = xpad.rearrange("p (b c d) -> p b c d", b=LB, c=CP, d=DP)

    # ---- X' ping-pong buffers ----
    xa = const.tile([128, xcols], F16)
    xb = const.tile([128, xcols], F16)
    bufs = [xa, xb]

    def xview(t):
        return t.rearrange("p (a b c j) -> p a b c j", a=la, b=LB, c=CP, j=J)

    # scatter map: s -> (j_lo, j_cnt, d_lo)  [d = 2j + s - 1]
    SMAP = {0: (1, 7, 1), 1: (0, 8, 0), 2: (0, 8, 1), 3: (0, 7, 2)}

    # ---- PE warm-up ----
    # Zero-matmuls issued while the first xpad/w0 DMA slices land: they keep
    # the tensor engine busy from t~0.1us so the p-state ramp (first 3us of a
    # continuous run at half clock) is spent inside the DMA shadow, and the
    # first real matmul starts at full speed with no idle gap.
    wz = const.tile([128, 128], F16)
    nc.vector.memset(wz[:], 0.0)
    wps = pp.tile([128, 512], F32, tag="ps")
    for _ in range(40):
        nc.tensor.matmul(wps[:, 0:128], wz[:], wz[:], start=True, stop=True)

    # Only the padding regions need zeroing (evictions write everything else
    # before it is read): c = 0 / 17 columns, plus the j-edge zeros of the
    # redundant d blocks (s=0 @ j=0 -> d=-1, s=3 @ j=7 -> d=16).  These run
    # on Pool during the L0 phase; the first consumer is layer 1's rhs.
    for t in bufs:
        v = xview(t)
        nc.gpsimd.memset(v[:, :, :, 0:1, :], 0.0)
        nc.gpsimd.memset(v[:, :, :, 17:18, :], 0.0)
        nc.gpsimd.memset(v[0:32, :, :, 1:17, 0:1], 0.0)
        nc.gpsimd.memset(v[96:128, :, :, 1:17, 7:8], 0.0)

    # ================= layer 0 =================
    def emit_l0_block(a0b):
        xn4 = xview(bufs[0])
        a_slope = slopes[0]
        for bc in range(8):          # b-pair chunks
            b0 = bc * 2
            ps = pp.tile([128, 512], F32, tag="ps")
            p4 = ps.rearrange("p (b c d) -> p b c d", b=2, c=16, d=16)
            for gi, (db, dc) in enumerate(G_L0):
                blo = max(b0, -db)
                bhi = min(b0 + 2, 16 - db)
                cb = (a0b * len(G_L0) + gi) * 128
                # K = 96: three d-shift blocks; rhs d-slice 1:17 uniform
                rhs = xp4[0:96, blo + db:bhi + db, dc + 1:dc + 17, 1:17]
                out = p4[:, blo - b0:bhi - b0, :, :]
                nc.tensor.matmul(out, w0t[0:96, cb:cb + 128], rhs,
                                 start=(gi == 0),
                                 stop=(gi == len(G_L0) - 1))
            # prelu in ONE ACT op (Lrelu, alpha=slope), then scatter the
            # chunk across DVE / GPSIMD / ACT so the scatter keeps up with
            # the PE during the L0 phase.
            sg = stg.tile([128, 512], F16, tag="l0st")
            nc.scalar.activation(sg[:], ps[:],
                                 mybir.ActivationFunctionType.Lrelu,
                                 alpha=a_slope)
            sg4 = sg.rearrange("p (b c d) -> p b c d", b=2, c=16, d=16)
            eng = 0
            for aj in range(4):
                a = a0b * 4 + aj
                for s in range(4):
                    jlo, jcnt, dlo = SMAP[s]
                    dst = xn4[32 * s:32 * s + 32, a, b0:b0 + 2, 1:17,
                              jlo:jlo + jcnt]
                    src = sg4[32 * aj:32 * aj + 32, :, :,
                              dlo:dlo + 2 * jcnt - 1:2]
                    if eng in (0, 3):
                        nc.vector.tensor_copy(dst, src)
                    elif eng in (1, 4):
                        nc.gpsimd.tensor_copy(dst, src)
                    else:
                        nc.scalar.copy(dst, src)
                    eng = (eng + 1) % 5

    # ================= layers 1..4 =================
    # M = 128 = (2 a-outs x 2 t x 32 ch); per (db,dc) the stationary for
    # source pattern p (src = a0 - 1 + p) is the contiguous da-block pair
    # (2-p, 3-p) of the bank [W(+1) | W(0) | W(-1)]; p=0/3 are M=64 halves.
    def emit_mid_group(l, g):
        xc4 = xview(bufs[(l + 1) % 2])
        xn4 = xview(bufs[l % 2])
        wt = wmt[l - 1]
        a_slope = slopes[l]
        a0 = 2 * g
        for chunk in range(4):
            b0 = 4 * chunk
            ps = pp.tile([128, 512], F32, tag="ps")
            mms = []
            for gi, (db, dc) in enumerate(G9):
                for p in (1, 2, 0, 3):   # full-M patterns first
                    src = a0 - 1 + p
                    if not (0 <= src < la):
                        continue
                    mms.append((gi, db, dc, p, src))
            nmm = len(mms)
            for idx, (gi, db, dc, p, src) in enumerate(mms):
                blo = max(b0, -db)
                bhi = min(b0 + 4, 16 - db)
                rhs = xc4[:, src, blo + db:bhi + db, dc + 1:dc + 17, :]
                cw = (blo - b0) * 128, (bhi - b0) * 128
                if p in (1, 2):
                    lhsT = wt[:, (gi * 3 + 2 - p) * 64:
                              (gi * 3 + 4 - p) * 64]
                    outv = ps[:, cw[0]:cw[1]]
                elif p == 0:   # only slot q=0 (out a0), da = -1
                    lhsT = wt[:, (gi * 3 + 2) * 64:(gi * 3 + 3) * 64]
                    outv = ps[0:64, cw[0]:cw[1]]
                else:          # p == 3: only slot q=1 (out a0+1), da = +1
                    lhsT = wt[:, (gi * 3 + 0) * 64:(gi * 3 + 1) * 64]
                    outv = ps[64:128, cw[0]:cw[1]]
                nc.tensor.matmul(outv, lhsT, rhs,
                                 start=(idx == 0), stop=(idx == nmm - 1))
            # prelu evictions: psum row block (64q + 32t) -> s' = t + 1;
            # one Lrelu ACT op per block; j-shift copies on DVE/GPSIMD
            for q in (0, 1):
                ap = a0 + q
                for t in (0, 1):
                    dst = xn4[32 * (t + 1):32 * (t + 2), ap,
                              b0:b0 + 4, 1:17, 0:8]
                    srcv = ps[64 * q + 32 * t:64 * q + 32 * t + 32, :]
                    nc.scalar.activation(
                        dst, srcv, mybir.ActivationFunctionType.Lrelu,
                        alpha=a_slope)
                # j-shift copies: s'=3 <- s'=1 (j+1) ; s'=0 <- s'=2 (j-1)
                nc.vector.tensor_copy(
                    xn4[96:128, ap, b0:b0 + 4, 1:17, 0:7],
                    xn4[32:64, ap, b0:b0 + 4, 1:17, 1:8])
                nc.gpsimd.tensor_copy(
                    xn4[0:32, ap, b0:b0 + 4, 1:17, 1:8],
                    xn4[64:96, ap, b0:b0 + 4, 1:17, 0:7])

    # Emission schedule: interleave layer-1 groups into the L0 tail (their
    # sources become ready after 1-3 L0 a-blocks), so the PE stays busy while
    # the L0 scatter (eviction-bound phase) drains.  Group g of layer 1 reads
    # sources 2g-1..2g+2, i.e. needs L0 a-blocks up to (2g+2)//4.
    emit_l0_block(0)
    emit_l0_block(1)
    emit_mid_group(1, 0)
    emit_mid_group(1, 1)
    emit_l0_block(2)
    emit_mid_group(1, 2)
    emit_mid_group(1, 3)
    emit_mid_group(1, 4)
    emit_l0_block(3)
    for g in range(5, 8):
        emit_mid_group(1, g)
    for l in range(2, 5):
        for g in range(8):
            emit_mid_group(l, g)

    # ================= layer 5 (a-banded, M=64 = 16a x 2o x 2t) =================
    xc4 = xview(bufs[0])
    for chunk in range(4):
        b0 = 4 * chunk
        psf = pp.tile([128, 512], F32, tag="ps")
        ps = psf[0:64, :]
        mms = []
        for gi, (db, dc) in enumerate(G9):
            for src in range(la):
                mms.append((gi, db, dc, src))
        nmm = len(mms)
        for idx, (gi, db, dc, src) in enumerate(mms):
            blo = max(b0, -db)
            bhi = min(b0 + 4, 16 - db)
            rhs = xc4[:, src, blo + db:bhi + db, dc + 1:dc + 17, :]
            lhsT = w5t[:, (gi * la + src) * 64:(gi * la + src) * 64 + 64]
            outv = ps[:, (blo - b0) * 128:(bhi - b0) * 128]
            nc.tensor.matmul(outv, lhsT, rhs,
                             start=(idx == 0), stop=(idx == nmm - 1))
        st = stg.tile([64, 512], F32, tag="stg")
        nc.vector.tensor_copy(st[:], ps[:])
        nc.sync.dma_start(out_d[:, chunk * 512:chunk * 512 + 512], st[:])


_CACHE = {}
LAST_RESULT = None


def _build(la, slopes):
    key = (la, tuple(slopes))
    if key not in _CACHE:
        nc = bacc.Bacc("TRN2")
        with tile.TileContext(nc) as tc:
            _conv_kernel(tc, la, slopes)
        nc.compile()
        _CACHE[key] = nc
    return _CACHE[key]


def kernel(x, k0, k1, k2, k3, k4, k5, slopes):
    x = np.asarray(x, np.float32)
    n, _, la = x.shape[:3]
    slopes_f = [float(s) for s in np.asarray(slopes, np.float32)]
    ws = _pack_weights((k0, k1, k2, k3, k4, k5), la)
    nc = _build(la, slopes_f)

    in_maps = []
    for i in range(n):
        m = {"xpad": _pack_x(x[i], la), "w0": ws[0], "w5": ws[5]}
        for l in (1, 2, 3, 4):
            m[f"w{l}"] = ws[l]
        in_maps.append(m)

    res = bass_utils.run_bass_kernel_spmd(nc, in_maps,
                                          core_ids=list(range(n)))
    global LAST_RESULT
    LAST_RESULT = res
    outs = []
    for i in range(n):
        od = res.results[i]["out_dec"].reshape(la, 2, 2, 16, 16, 8)
        # rows (a, o, t) x cols (b, c, j) -> [o, a, b, c, d = 2j + t]
        o = np.transpose(od, (1, 0, 3, 4, 5, 2)).reshape(2, la, 16, 16, 16)
        outs.append(o)
    return np.stack(outs).astype(np.float32)


# revision 5
# speedup vs baseline: 1.0507x; 1.0002x over previous
"""Trainium2 Bass kernel for a 6-layer 4D CNN (3^4 SAME convs + PReLU) — v2.

Problem: x (8, 2, 16,16,16,16) -> 6 conv layers, channels 2->32->32->32->32
->32->2, PReLU (scalar slope) after the first five convs.  Data-parallel over
batch N=8 across 8 cores.

v2 changes vs the v1 baseline (1824 us modeled):
  * Mid layers (1..4) use full M=128 matmuls: M = (2 a-outs x 2 d-outs x 32
    ch).  Each 2-wide a-output group accumulates streams from its <=4 source
    a-columns; the stationary for a source is a contiguous 128-col slice of a
    per-(db,dc) bank laid out [W(da=+1) | W(da=0) | W(da=-1)], so the
    (da_for_slot0, da_for_slot1) pair needed by each source pattern is one
    affine AP.  End sources use M=64 half matmuls (psum row offset).
    30 streams per (db,dc) pair per layer vs 46 at M=64: 812k -> 530k PE cyc.
  * Layer 5 uses an a-banded stationary: M = 64 = (16 a x 2 out-ch x 2
    d-outs); each source a streams once per (db,dc) into a +-1 band of psum
    rows (psum partition offset).  One [64,512] psum tile per b-chunk
    accumulates all 144 matmuls (zero-stationary matmul opens the group).
    812k/4-packed -> 283k PE cycles.
  * L0 input (d-shift replicated, padded, fp16) is packed on the host and
    DMA'd directly; the fp32 staging buffer and on-chip conversion are gone.
  * d-axis decimated activation layout X'[32*s + ch, (a, b, c_pad, j)] with
    d = 2*j + s - 1 is unchanged from v1, as are L0's a-partition scheme,
    PReLU eviction (ACT relu-scale + DVE scalar_tensor_tensor), and the
    j-shift copies that restore the redundant s=0/3 blocks.
"""

import sys

import numpy as np

for _p in ("/opt/trn_rl_repo", "/root/.axon_site/_ro/trn_rl_repo"):
    if _p not in sys.path:
        sys.path.append(_p)

import concourse.bass as bass  # noqa: E402
import concourse.mybir as mybir  # noqa: E402
import concourse.tile as tile  # noqa: E402
from concourse import bacc, bass_utils  # noqa: E402
from concourse._compat import with_exitstack  # noqa: E402

F32 = mybir.dt.float32
F16 = mybir.dt.float16

LB = 16
CP = 18   # padded c axis (c in -1..16)
DP = 18   # padded d axis in x_pad
J = 8     # d//2

# tap order: db=0 first so the first matmul of every psum group covers the
# full b-window (start=True zeroes the whole tile)
G9 = [(db, dc) for db in (0, -1, 1) for dc in (0, -1, 1)]
G_L0 = [(db, dc) for db in (0, -1, 1) for dc in (0, -1, 1)]


def _pack_weights(ks, la):
    """Host-side packing of conv kernels into stationary matrices (fp16)."""
    na4 = la // 4
    k0, k1, k2, k3, k4, k5 = [np.asarray(k, np.float32) for k in ks]

    # L0: W0[32*rg + i*la + a_in, (a0b*9 + g)*128 + a_j*32 + o]  (a0b-major
    # so the DMA can be sliced per a0b block)
    # K = 96 = 3 d-shift blocks (rg -> dd = rg - 1) x (2 ch x la a_in, banded)
    w0 = np.zeros((128, len(G_L0) * na4 * 128), np.float32)
    for gi, (db, dc) in enumerate(G_L0):
        for a0b in range(na4):
            cb = (a0b * len(G_L0) + gi) * 128
            for rg in range(3):
                for aj in range(4):
                    for da in (-1, 0, 1):
                        ain = a0b * 4 + aj + da
                        if not (0 <= ain < la):
                            continue
                        for i in range(2):
                            w0[32 * rg + i * la + ain,
                               cb + aj * 32:cb + aj * 32 + 32] = \
                                k0[:, i, da + 1, db + 1, dc + 1, rg]

    # mid layers: per (db,dc) bank of three da blocks ordered (+1, 0, -1);
    # block col = t*32 + o; W(da)[32*s + i, t*32 + o] = k[o,i,da,db,dc,s-t]
    def pack_mid(k):
        w = np.zeros((128, 9 * 3 * 64), np.float32)
        for gi, (db, dc) in enumerate(G9):
            for bi, da in enumerate((1, 0, -1)):
                cb = (gi * 3 + bi) * 64
                for s in range(4):
                    for t in range(2):
                        if 0 <= s - t <= 2:
                            w[32 * s:32 * s + 32,
                              cb + t * 32:cb + t * 32 + 32] = \
                                k[:, :, da + 1, db + 1, dc + 1, s - t].T
        return w

    # L5 banded, full-M stationaries (psum row offsets must be 32-aligned):
    # per (gi, src) a [128, 64] matrix, col = 4*a' + 2*o + t nonzero only for
    # a' in the +-1 band of src (da = src - a').
    w5 = np.zeros((128, 9 * la * 64), np.float32)
    for gi, (db, dc) in enumerate(G9):
        for src in range(la):
            cb = (gi * la + src) * 64
            for ap_ in (src - 1, src, src + 1):
                if not (0 <= ap_ < la):
                    continue
                da = src - ap_
                for s in range(4):
                    for t in range(2):
                        if 0 <= s - t <= 2:
                            for o in range(2):
                                w5[32 * s:32 * s + 32,
                                   cb + 4 * ap_ + 2 * o + t] = \
                                    k5[o, :, da + 1, db + 1, dc + 1, s - t]

    return ([w0.astype(np.float16)] +
            [pack_mid(k).astype(np.float16) for k in (k1, k2, k3, k4)] +
            [w5.astype(np.float16)])


def _pack_x(x1, la):
    """One sample (2, la, 16, 16, 16) -> padded fp16 [128, 16*18*18].

    partition = 32*rg + i*la + a holds x[i, a] shifted in d by dd = rg - 1;
    b unpadded, c padded to 18 (data at 1..16), d padded to 18.
    """
    xp = np.zeros((128, LB, CP, DP), np.float16)
    x1 = np.asarray(x1, np.float16)
    for rg in range(3):
        for i in range(2):
            xp[32 * rg + i * la:32 * rg + i * la + la, :, 1:17,
               2 - rg:18 - rg] = x1[i]
    return np.ascontiguousarray(xp.reshape(128, LB * CP * DP))


@with_exitstack
def _conv_kernel(ctx, tc, la, slopes):
    """Emit the full 6-layer conv program. slopes: python floats len 5."""
    nc = tc.nc
    na4 = la // 4
    xcols = la * LB * CP * J

    xpad_d = nc.dram_tensor("xpad", [128, LB * CP * DP], F16,
                            kind="ExternalInput")
    w0_d = nc.dram_tensor("w0", [128, len(G_L0) * na4 * 128],
                          F16, kind="ExternalInput")
    wmid_d = [nc.dram_tensor(f"w{l}", [128, 9 * 3 * 64], F16,
                             kind="ExternalInput") for l in (1, 2, 3, 4)]
    w5_d = nc.dram_tensor("w5", [128, 9 * la * 64], F16, kind="ExternalInput")
    out_d = nc.dram_tensor("out_dec", [64, la * 128], F32,
                           kind="ExternalOutput")

    const = ctx.enter_context(tc.tile_pool(name="const", bufs=1))
    pp = ctx.enter_context(tc.tile_pool(name="ps", bufs=8, space="PSUM"))
    stg = ctx.enter_context(tc.tile_pool(name="stg", bufs=4))

    # ---- load weights + padded input ----
    # xpad and w0 are DMA'd in b-/a0b-slices so layer 0's first chunks are
    # gated by ~0.6MB of DMA instead of the full 2.5MB.
    xpad = const.tile([128, LB * CP * DP], F16)
    xpv = xpad.rearrange("p (b cd) -> p b cd", b=LB, cd=CP * DP)
    xdv = xpad_d.rearrange("p (b cd) -> p b cd", b=LB, cd=CP * DP)
    w0t = const.tile([128, w0_d.shape[1]], F16)
    # interleave xpad/w0 slices so the first L0 chunk is gated by only the
    # first xpad b-slice + first w0 block (~0.6MB), not the whole 2.5MB
    for a0b in range(na4):
        b0 = a0b * 4
        nc.sync.dma_start(xpv[:, b0:b0 + 4, :], xdv[:, b0:b0 + 4, :])
        cb = a0b * 9 * 128
        nc.sync.dma_start(w0t[:, cb:cb + 9 * 128], w0_d[:, cb:cb + 9 * 128])
    wmt = []
    for wd in wmid_d:
        t = const.tile([128, 9 * 3 * 64], F16, tag=wd.name)
        nc.sync.dma_start(t[:], wd[:])
        wmt.append(t)
    w5t = const.tile([128, 9 * la * 64], F16)
    nc.sync.dma_start(w5t[:], w5_d[:])

    xp4 = xpad.rearrange("p (b c d) -> p b c d", b=LB, c=CP, d=DP)

    # ---- X' ping-pong buffers ----
    xa = const.tile([128, xcols], F16)
    xb = const.tile([128, xcols], F16)
    bufs = [xa, xb]

    def xview(t):
        return t.rearrange("p (a b c j) -> p a b c j", a=la, b=LB, c=CP, j=J)

    # scatter map: s -> (j_lo, j_cnt, d_lo)  [d = 2j + s - 1]
    SMAP = {0: (1, 7, 1), 1: (0, 8, 0), 2: (0, 8, 1), 3: (0, 7, 2)}

    # ---- PE warm-up ----
    # Zero-matmuls issued while the first xpad/w0 DMA slices land: they keep
    # the tensor engine busy from t~0.1us so the p-state ramp (first 3us of a
    # continuous run at half clock) is spent inside the DMA shadow, and the
    # first real matmul starts at full speed with no idle gap.
    wz = const.tile([128, 128], F16)
    nc.vector.memset(wz[:], 0.0)
    wps = pp.tile([128, 512], F32, tag="ps")
    for _ in range(40):
        nc.tensor.matmul(wps[:, 0:128], wz[:], wz[:], start=True, stop=True)

    # Only the padding regions need zeroing (evictions write everything else
    # before it is read): c = 0 / 17 columns, plus the j-edge zeros of the
    # redundant d blocks (s=0 @ j=0 -> d=-1, s=3 @ j=7 -> d=16).  These run
    # on Pool during the L0 phase; the first consumer is layer 1's rhs.
    for t in bufs:
        v = xview(t)
        nc.gpsimd.memset(v[:, :, :, 0:1, :], 0.0)
        nc.gpsimd.memset(v[:, :, :, 17:18, :], 0.0)
        nc.gpsimd.memset(v[0:32, :, :, 1:17, 0:1], 0.0)
        nc.gpsimd.memset(v[96:128, :, :, 1:17, 7:8], 0.0)

    # ================= layer 0 =================
    def emit_l0_chunk(a0b, bc):
        xn4 = xview(bufs[0])
        a_slope = slopes[0]
        if True:
            b0 = bc * 2
            ps = pp.tile([128, 512], F32, tag="ps")
            p4 = ps.rearrange("p (b c d) -> p b c d", b=2, c=16, d=16)
            for gi, (db, dc) in enumerate(G_L0):
                blo = max(b0, -db)
                bhi = min(b0 + 2, 16 - db)
                cb = (a0b * len(G_L0) + gi) * 128
                # K = 96: three d-shift blocks; rhs d-slice 1:17 uniform
                rhs = xp4[0:96, blo + db:bhi + db, dc + 1:dc + 17, 1:17]
                out = p4[:, blo - b0:bhi - b0, :, :]
                nc.tensor.matmul(out, w0t[0:96, cb:cb + 128], rhs,
                                 start=(gi == 0),
                                 stop=(gi == len(G_L0) - 1))
            # prelu in ONE ACT op (Lrelu, alpha=slope), then scatter the
            # chunk across DVE / GPSIMD / ACT so the scatter keeps up with
            # the PE during the L0 phase.
            sg = stg.tile([128, 512], F16, tag="l0st")
            nc.scalar.activation(sg[:], ps[:],
                                 mybir.ActivationFunctionType.Lrelu,
                                 alpha=a_slope)
            sg4 = sg.rearrange("p (b c d) -> p b c d", b=2, c=16, d=16)
            eng = 0
            for aj in range(4):
                a = a0b * 4 + aj
                for s in range(4):
                    jlo, jcnt, dlo = SMAP[s]
                    dst = xn4[32 * s:32 * s + 32, a, b0:b0 + 2, 1:17,
                              jlo:jlo + jcnt]
                    src = sg4[32 * aj:32 * aj + 32, :, :,
                              dlo:dlo + 2 * jcnt - 1:2]
                    if eng in (0, 3):
                        nc.vector.tensor_copy(dst, src)
                    elif eng in (1, 4):
                        nc.gpsimd.tensor_copy(dst, src)
                    else:
                        nc.scalar.copy(dst, src)
                    eng = (eng + 1) % 5

    # ================= layers 1..4 =================
    # M = 128 = (2 a-outs x 2 t x 32 ch); per (db,dc) the stationary for
    # source pattern p (src = a0 - 1 + p) is the contiguous da-block pair
    # (2-p, 3-p) of the bank [W(+1) | W(0) | W(-1)]; p=0/3 are M=64 halves.
    def emit_mid_chunk(l, g, chunk):
        xc4 = xview(bufs[(l + 1) % 2])
        xn4 = xview(bufs[l % 2])
        wt = wmt[l - 1]
        a_slope = slopes[l]
        a0 = 2 * g
        if True:
            b0 = 4 * chunk
            ps = pp.tile([128, 512], F32, tag="ps")
            mms = []
            for gi, (db, dc) in enumerate(G9):
                for p in (1, 2, 0, 3):   # full-M patterns first
                    src = a0 - 1 + p
                    if not (0 <= src < la):
                        continue
                    mms.append((gi, db, dc, p, src))
            nmm = len(mms)
            for idx, (gi, db, dc, p, src) in enumerate(mms):
                blo = max(b0, -db)
                bhi = min(b0 + 4, 16 - db)
                rhs = xc4[:, src, blo + db:bhi + db, dc + 1:dc + 17, :]
                cw = (blo - b0) * 128, (bhi - b0) * 128
                if p in (1, 2):
                    lhsT = wt[:, (gi * 3 + 2 - p) * 64:
                              (gi * 3 + 4 - p) * 64]
                    outv = ps[:, cw[0]:cw[1]]
                elif p == 0:   # only slot q=0 (out a0), da = -1
                    lhsT = wt[:, (gi * 3 + 2) * 64:(gi * 3 + 3) * 64]
                    outv = ps[0:64, cw[0]:cw[1]]
                else:          # p == 3: only slot q=1 (out a0+1), da = +1
                    lhsT = wt[:, (gi * 3 + 0) * 64:(gi * 3 + 1) * 64]
                    outv = ps[64:128, cw[0]:cw[1]]
                nc.tensor.matmul(outv, lhsT, rhs,
                                 start=(idx == 0), stop=(idx == nmm - 1))
            # prelu evictions: psum row block (64q + 32t) -> s' = t + 1;
            # one Lrelu ACT op per block; j-shift copies on DVE/GPSIMD
            for q in (0, 1):
                ap = a0 + q
                for t in (0, 1):
                    dst = xn4[32 * (t + 1):32 * (t + 2), ap,
                              b0:b0 + 4, 1:17, 0:8]
                    srcv = ps[64 * q + 32 * t:64 * q + 32 * t + 32, :]
                    nc.scalar.activation(
                        dst, srcv, mybir.ActivationFunctionType.Lrelu,
                        alpha=a_slope)
                # j-shift copies: s'=3 <- s'=1 (j+1) ; s'=0 <- s'=2 (j-1)
                nc.vector.tensor_copy(
                    xn4[96:128, ap, b0:b0 + 4, 1:17, 0:7],
                    xn4[32:64, ap, b0:b0 + 4, 1:17, 1:8])
                nc.gpsimd.tensor_copy(
                    xn4[0:32, ap, b0:b0 + 4, 1:17, 1:8],
                    xn4[64:96, ap, b0:b0 + 4, 1:17, 0:7])

    # Emission schedule: interleave layer-1 groups into the L0 tail (their
    # sources become ready after 1-3 L0 a-blocks), so the PE stays busy while
    # the L0 scatter (eviction-bound phase) drains.  Group g of layer 1 reads
    # sources 2g-1..2g+2, i.e. needs L0 a-blocks up to (2g+2)//4.
    # Chunk-level interleave of layer 1 into the L0 phase: a layer-1 chunk
    # (g, k) is inserted once every L0 chunk it reads (source a-blocks x
    # b-chunks) has been emitted for >= LAG further L0 chunks (so its scatter
    # has drained and the in-order PE queue never head-of-line blocks).
    LAG = 3
    ready_at = {}
    emitted = set()
    idx = 0
    for a0b in range(na4):
        for bc in range(8):
            emit_l0_chunk(a0b, bc)
            idx += 1
            for g in range(8):
                ablo = max(0, 2 * g - 1) // 4
                abhi = min(la - 1, 2 * g + 2) // 4
                for k in range(4):
                    if (g, k) in ready_at:
                        continue
                    need = [(ab, c) for ab in range(ablo, abhi + 1)
                            for c in range(max(0, 2 * k - 1),
                                           min(7, 2 * k + 2) + 1)]
                    if all(n[0] < a0b or (n[0] == a0b and n[1] <= bc)
                           for n in need):
                        ready_at[(g, k)] = idx
            # emit at most one lagged-ready layer-1 chunk per L0 chunk, and
            # none before idx 14 (the w1 weight DMA lands ~12us in)
            if idx >= 14:
                for (g, k), r in sorted(ready_at.items(), key=lambda kv: kv[1]):
                    if (g, k) not in emitted and idx - r >= LAG:
                        emit_mid_chunk(1, g, k)
                        emitted.add((g, k))
                        break
    for g in range(8):
        for k in range(4):
            if (g, k) not in emitted:
                emit_mid_chunk(1, g, k)
    for l in range(2, 5):
        for g in range(8):
            for k in range(4):
                emit_mid_chunk(l, g, k)

    # ================= layer 5 (a-banded, M=64 = 16a x 2o x 2t) =================
    xc4 = xview(bufs[0])
    for chunk in range(4):
        b0 = 4 * chunk
        psf = pp.tile([128, 512], F32, tag="ps")
        ps = psf[0:64, :]
        mms = []
        for gi, (db, dc) in enumerate(G9):
            for src in range(la):
                mms.append((gi, db, dc, src))
        nmm = len(mms)
        for idx, (gi, db, dc, src) in enumerate(mms):
            blo = max(b0, -db)
            bhi = min(b0 + 4, 16 - db)
            rhs = xc4[:, src, blo + db:bhi + db, dc + 1:dc + 17, :]
            lhsT = w5t[:, (gi * la + src) * 64:(gi * la + src) * 64 + 64]
            outv = ps[:, (blo - b0) * 128:(bhi - b0) * 128]
            nc.tensor.matmul(outv, lhsT, rhs,
                             start=(idx == 0), stop=(idx == nmm - 1))
        st = stg.tile([64, 512], F32, tag="stg")
        nc.vector.tensor_copy(st[:], ps[:])
        nc.sync.dma_start(out_d[:, chunk * 512:chunk * 512 + 512], st[:])


_CACHE = {}
LAST_RESULT = None


def _build(la, slopes):
    key = (la, tuple(slopes))
    if key not in _CACHE:
        nc = bacc.Bacc("TRN2")
        with tile.TileContext(nc) as tc:
            _conv_kernel(tc, la, slopes)
        nc.compile()
        _CACHE[key] = nc
    return _CACHE[key]


def kernel(x, k0, k1, k2, k3, k4, k5, slopes):
    x = np.asarray(x, np.float32)
    n, _, la = x.shape[:3]
    slopes_f = [float(s) for s in np.asarray(slopes, np.float32)]
    ws = _pack_weights((k0, k1, k2, k3, k4, k5), la)
    nc = _build(la, slopes_f)

    in_maps = []
    for i in range(n):
        m = {"xpad": _pack_x(x[i], la), "w0": ws[0], "w5": ws[5]}
        for l in (1, 2, 3, 4):
            m[f"w{l}"] = ws[l]
        in_maps.append(m)

    res = bass_utils.run_bass_kernel_spmd(nc, in_maps,
                                          core_ids=list(range(n)))
    global LAST_RESULT
    LAST_RESULT = res
    outs = []
    for i in range(n):
        od = res.results[i]["out_dec"].reshape(la, 2, 2, 16, 16, 8)
        # rows (a, o, t) x cols (b, c, j) -> [o, a, b, c, d = 2j + t]
        o = np.transpose(od, (1, 0, 3, 4, 5, 2)).reshape(2, la, 16, 16, 16)
        outs.append(o)
    return np.stack(outs).astype(np.float32)


# revision 6
# speedup vs baseline: 1.0510x; 1.0003x over previous
"""Trainium2 Bass kernel for a 6-layer 4D CNN (3^4 SAME convs + PReLU) — v2.

Problem: x (8, 2, 16,16,16,16) -> 6 conv layers, channels 2->32->32->32->32
->32->2, PReLU (scalar slope) after the first five convs.  Data-parallel over
batch N=8 across 8 cores.

v2 changes vs the v1 baseline (1824 us modeled):
  * Mid layers (1..4) use full M=128 matmuls: M = (2 a-outs x 2 d-outs x 32
    ch).  Each 2-wide a-output group accumulates streams from its <=4 source
    a-columns; the stationary for a source is a contiguous 128-col slice of a
    per-(db,dc) bank laid out [W(da=+1) | W(da=0) | W(da=-1)], so the
    (da_for_slot0, da_for_slot1) pair needed by each source pattern is one
    affine AP.  End sources use M=64 half matmuls (psum row offset).
    30 streams per (db,dc) pair per layer vs 46 at M=64: 812k -> 530k PE cyc.
  * Layer 5 uses an a-banded stationary: M = 64 = (16 a x 2 out-ch x 2
    d-outs); each source a streams once per (db,dc) into a +-1 band of psum
    rows (psum partition offset).  One [64,512] psum tile per b-chunk
    accumulates all 144 matmuls (zero-stationary matmul opens the group).
    812k/4-packed -> 283k PE cycles.
  * L0 input (d-shift replicated, padded, fp16) is packed on the host and
    DMA'd directly; the fp32 staging buffer and on-chip conversion are gone.
  * d-axis decimated activation layout X'[32*s + ch, (a, b, c_pad, j)] with
    d = 2*j + s - 1 is unchanged from v1, as are L0's a-partition scheme,
    PReLU eviction (ACT relu-scale + DVE scalar_tensor_tensor), and the
    j-shift copies that restore the redundant s=0/3 blocks.
"""

import sys

import numpy as np

for _p in ("/opt/trn_rl_repo", "/root/.axon_site/_ro/trn_rl_repo"):
    if _p not in sys.path:
        sys.path.append(_p)

import concourse.bass as bass  # noqa: E402
import concourse.mybir as mybir  # noqa: E402
import concourse.tile as tile  # noqa: E402
from concourse import bacc, bass_utils  # noqa: E402
from concourse._compat import with_exitstack  # noqa: E402

F32 = mybir.dt.float32
F16 = mybir.dt.float16

LB = 16
CP = 18   # padded c axis (c in -1..16)
DP = 18   # padded d axis in x_pad
J = 8     # d//2

# tap order: db=0 first so the first matmul of every psum group covers the
# full b-window (start=True zeroes the whole tile)
G9 = [(db, dc) for db in (0, -1, 1) for dc in (0, -1, 1)]
G_L0 = [(db, dc) for db in (0, -1, 1) for dc in (0, -1, 1)]


def _pack_weights(ks, la):
    """Host-side packing of conv kernels into stationary matrices (fp16)."""
    na4 = la // 4
    k0, k1, k2, k3, k4, k5 = [np.asarray(k, np.float32) for k in ks]

    # L0: W0[32*rg + i*la + a_in, (a0b*9 + g)*128 + a_j*32 + o]  (a0b-major
    # so the DMA can be sliced per a0b block)
    # K = 96 = 3 d-shift blocks (rg -> dd = rg - 1) x (2 ch x la a_in, banded)
    w0 = np.zeros((128, len(G_L0) * na4 * 128), np.float32)
    for gi, (db, dc) in enumerate(G_L0):
        for a0b in range(na4):
            cb = (a0b * len(G_L0) + gi) * 128
            for rg in range(3):
                for aj in range(4):
                    for da in (-1, 0, 1):
                        ain = a0b * 4 + aj + da
                        if not (0 <= ain < la):
                            continue
                        for i in range(2):
                            w0[32 * rg + i * la + ain,
                               cb + aj * 32:cb + aj * 32 + 32] = \
                                k0[:, i, da + 1, db + 1, dc + 1, rg]

    # mid layers: per (db,dc) bank of three da blocks ordered (+1, 0, -1);
    # block col = t*32 + o; W(da)[32*s + i, t*32 + o] = k[o,i,da,db,dc,s-t]
    def pack_mid(k):
        w = np.zeros((128, 9 * 3 * 64), np.float32)
        for gi, (db, dc) in enumerate(G9):
            for bi, da in enumerate((1, 0, -1)):
                cb = (gi * 3 + bi) * 64
                for s in range(4):
                    for t in range(2):
                        if 0 <= s - t <= 2:
                            w[32 * s:32 * s + 32,
                              cb + t * 32:cb + t * 32 + 32] = \
                                k[:, :, da + 1, db + 1, dc + 1, s - t].T
        return w

    # L5 banded, full-M stationaries (psum row offsets must be 32-aligned):
    # per (gi, src) a [128, 64] matrix, col = 4*a' + 2*o + t nonzero only for
    # a' in the +-1 band of src (da = src - a').
    w5 = np.zeros((128, 9 * la * 64), np.float32)
    for gi, (db, dc) in enumerate(G9):
        for src in range(la):
            cb = (gi * la + src) * 64
            for ap_ in (src - 1, src, src + 1):
                if not (0 <= ap_ < la):
                    continue
                da = src - ap_
                for s in range(4):
                    for t in range(2):
                        if 0 <= s - t <= 2:
                            for o in range(2):
                                w5[32 * s:32 * s + 32,
                                   cb + 4 * ap_ + 2 * o + t] = \
                                    k5[o, :, da + 1, db + 1, dc + 1, s - t]

    return ([w0.astype(np.float16)] +
            [pack_mid(k).astype(np.float16) for k in (k1, k2, k3, k4)] +
            [w5.astype(np.float16)])


def _pack_x(x1, la):
    """One sample (2, la, 16, 16, 16) -> padded fp16 [128, 16*18*18].

    partition = 32*rg + i*la + a holds x[i, a] shifted in d by dd = rg - 1;
    b unpadded, c padded to 18 (data at 1..16), d padded to 18.
    """
    xp = np.zeros((128, LB, CP, DP), np.float16)
    x1 = np.asarray(x1, np.float16)
    for rg in range(3):
        for i in range(2):
            xp[32 * rg + i * la:32 * rg + i * la + la, :, 1:17,
               2 - rg:18 - rg] = x1[i]
    return np.ascontiguousarray(xp.reshape(128, LB * CP * DP))


@with_exitstack
def _conv_kernel(ctx, tc, la, slopes):
    """Emit the full 6-layer conv program. slopes: python floats len 5."""
    nc = tc.nc
    na4 = la // 4
    xcols = la * LB * CP * J

    xpad_d = nc.dram_tensor("xpad", [128, LB * CP * DP], F16,
                            kind="ExternalInput")
    w0_d = nc.dram_tensor("w0", [128, len(G_L0) * na4 * 128],
                          F16, kind="ExternalInput")
    wmid_d = [nc.dram_tensor(f"w{l}", [128, 9 * 3 * 64], F16,
                             kind="ExternalInput") for l in (1, 2, 3, 4)]
    w5_d = nc.dram_tensor("w5", [128, 9 * la * 64], F16, kind="ExternalInput")
    out_d = nc.dram_tensor("out_dec", [64, la * 128], F32,
                           kind="ExternalOutput")

    const = ctx.enter_context(tc.tile_pool(name="const", bufs=1))
    pp = ctx.enter_context(tc.tile_pool(name="ps", bufs=8, space="PSUM"))
    stg = ctx.enter_context(tc.tile_pool(name="stg", bufs=4))

    # ---- load weights + padded input ----
    # xpad and w0 are DMA'd in b-/a0b-slices so layer 0's first chunks are
    # gated by ~0.6MB of DMA instead of the full 2.5MB.
    xpad = const.tile([128, LB * CP * DP], F16)
    xpv = xpad.rearrange("p (b cd) -> p b cd", b=LB, cd=CP * DP)
    xdv = xpad_d.rearrange("p (b cd) -> p b cd", b=LB, cd=CP * DP)
    w0t = const.tile([128, w0_d.shape[1]], F16)
    # interleave xpad/w0 slices so the first L0 chunk is gated by only the
    # first xpad b-slice + first w0 block (~0.6MB), not the whole 2.5MB
    for a0b in range(na4):
        b0 = a0b * 4
        nc.sync.dma_start(xpv[:, b0:b0 + 4, :], xdv[:, b0:b0 + 4, :])
        cb = a0b * 9 * 128
        nc.sync.dma_start(w0t[:, cb:cb + 9 * 128], w0_d[:, cb:cb + 9 * 128])
    wmt = []
    for wd in wmid_d:
        t = const.tile([128, 9 * 3 * 64], F16, tag=wd.name)
        nc.sync.dma_start(t[:], wd[:])
        wmt.append(t)
    w5t = const.tile([128, 9 * la * 64], F16)
    nc.sync.dma_start(w5t[:], w5_d[:])

    xp4 = xpad.rearrange("p (b c d) -> p b c d", b=LB, c=CP, d=DP)

    # ---- X' ping-pong buffers ----
    xa = const.tile([128, xcols], F16)
    xb = const.tile([128, xcols], F16)
    bufs = [xa, xb]

    def xview(t):
        return t.rearrange("p (a b c j) -> p a b c j", a=la, b=LB, c=CP, j=J)

    # scatter map: s -> (j_lo, j_cnt, d_lo)  [d = 2j + s - 1]
    SMAP = {0: (1, 7, 1), 1: (0, 8, 0), 2: (0, 8, 1), 3: (0, 7, 2)}

    # ---- PE warm-up ----
    # Zero-matmuls issued while the first xpad/w0 DMA slices land: they keep
    # the tensor engine busy from t~0.1us so the p-state ramp (first 3us of a
    # continuous run at half clock) is spent inside the DMA shadow, and the
    # first real matmul starts at full speed with no idle gap.
    wz = const.tile([128, 128], F16)
    nc.vector.memset(wz[:], 0.0)
    wps = pp.tile([128, 512], F32, tag="ps")
    for _ in range(40):
        nc.tensor.matmul(wps[:, 0:128], wz[:], wz[:], start=True, stop=True)

    # Only the padding regions need zeroing (evictions write everything else
    # before it is read): c = 0 / 17 columns, plus the j-edge zeros of the
    # redundant d blocks (s=0 @ j=0 -> d=-1, s=3 @ j=7 -> d=16).  These run
    # on Pool during the L0 phase; the first consumer is layer 1's rhs.
    for t in bufs:
        v = xview(t)
        nc.gpsimd.memset(v[:, :, :, 0:1, :], 0.0)
        nc.gpsimd.memset(v[:, :, :, 17:18, :], 0.0)
        nc.gpsimd.memset(v[0:32, :, :, 1:17, 0:1], 0.0)
        nc.gpsimd.memset(v[96:128, :, :, 1:17, 7:8], 0.0)

    # ================= layer 0 =================
    def emit_l0_chunk(a0b, bc):
        xn4 = xview(bufs[0])
        a_slope = slopes[0]
        if True:
            b0 = bc * 2
            ps = pp.tile([128, 512], F32, tag="ps")
            p4 = ps.rearrange("p (b c d) -> p b c d", b=2, c=16, d=16)
            for gi, (db, dc) in enumerate(G_L0):
                blo = max(b0, -db)
                bhi = min(b0 + 2, 16 - db)
                cb = (a0b * len(G_L0) + gi) * 128
                # K = 96: three d-shift blocks; rhs d-slice 1:17 uniform
                rhs = xp4[0:96, blo + db:bhi + db, dc + 1:dc + 17, 1:17]
                out = p4[:, blo - b0:bhi - b0, :, :]
                nc.tensor.matmul(out, w0t[0:96, cb:cb + 128], rhs,
                                 start=(gi == 0),
                                 stop=(gi == len(G_L0) - 1))
            # prelu in ONE ACT op (Lrelu, alpha=slope), then scatter the
            # chunk across DVE / GPSIMD / ACT so the scatter keeps up with
            # the PE during the L0 phase.
            sg = stg.tile([128, 512], F16, tag="l0st")
            nc.scalar.activation(sg[:], ps[:],
                                 mybir.ActivationFunctionType.Lrelu,
                                 alpha=a_slope)
            sg4 = sg.rearrange("p (b c d) -> p b c d", b=2, c=16, d=16)
            eng = 0
            for aj in range(4):
                a = a0b * 4 + aj
                for s in range(4):
                    jlo, jcnt, dlo = SMAP[s]
                    dst = xn4[32 * s:32 * s + 32, a, b0:b0 + 2, 1:17,
                              jlo:jlo + jcnt]
                    src = sg4[32 * aj:32 * aj + 32, :, :,
                              dlo:dlo + 2 * jcnt - 1:2]
                    if eng in (0, 3):
                        nc.vector.tensor_copy(dst, src)
                    elif eng in (1, 4):
                        nc.gpsimd.tensor_copy(dst, src)
                    else:
                        nc.scalar.copy(dst, src)
                    eng = (eng + 1) % 5

    # ================= layers 1..4 =================
    # M = 128 = (2 a-outs x 2 t x 32 ch); per (db,dc) the stationary for
    # source pattern p (src = a0 - 1 + p) is the contiguous da-block pair
    # (2-p, 3-p) of the bank [W(+1) | W(0) | W(-1)]; p=0/3 are M=64 halves.
    def emit_mid_chunk(l, g, chunk):
        xc4 = xview(bufs[(l + 1) % 2])
        xn4 = xview(bufs[l % 2])
        wt = wmt[l - 1]
        a_slope = slopes[l]
        a0 = 2 * g
        if True:
            b0 = 4 * chunk
            ps = pp.tile([128, 512], F32, tag="ps")
            mms = []
            for gi, (db, dc) in enumerate(G9):
                for p in (1, 2, 0, 3):   # full-M patterns first
                    src = a0 - 1 + p
                    if not (0 <= src < la):
                        continue
                    mms.append((gi, db, dc, p, src))
            nmm = len(mms)
            for idx, (gi, db, dc, p, src) in enumerate(mms):
                blo = max(b0, -db)
                bhi = min(b0 + 4, 16 - db)
                rhs = xc4[:, src, blo + db:bhi + db, dc + 1:dc + 17, :]
                cw = (blo - b0) * 128, (bhi - b0) * 128
                if p in (1, 2):
                    lhsT = wt[:, (gi * 3 + 2 - p) * 64:
                              (gi * 3 + 4 - p) * 64]
                    outv = ps[:, cw[0]:cw[1]]
                elif p == 0:   # only slot q=0 (out a0), da = -1
                    lhsT = wt[:, (gi * 3 + 2) * 64:(gi * 3 + 3) * 64]
                    outv = ps[0:64, cw[0]:cw[1]]
                else:          # p == 3: only slot q=1 (out a0+1), da = +1
                    lhsT = wt[:, (gi * 3 + 0) * 64:(gi * 3 + 1) * 64]
                    outv = ps[64:128, cw[0]:cw[1]]
                nc.tensor.matmul(outv, lhsT, rhs,
                                 start=(idx == 0), stop=(idx == nmm - 1))
            # prelu evictions: psum row block (64q + 32t) -> s' = t + 1;
            # one Lrelu ACT op per block; j-shift copies on DVE/GPSIMD
            for q in (0, 1):
                ap = a0 + q
                for t in (0, 1):
                    dst = xn4[32 * (t + 1):32 * (t + 2), ap,
                              b0:b0 + 4, 1:17, 0:8]
                    srcv = ps[64 * q + 32 * t:64 * q + 32 * t + 32, :]
                    nc.scalar.activation(
                        dst, srcv, mybir.ActivationFunctionType.Lrelu,
                        alpha=a_slope)
                # j-shift copies: s'=3 <- s'=1 (j+1) ; s'=0 <- s'=2 (j-1)
                nc.vector.tensor_copy(
                    xn4[96:128, ap, b0:b0 + 4, 1:17, 0:7],
                    xn4[32:64, ap, b0:b0 + 4, 1:17, 1:8])
                nc.gpsimd.tensor_copy(
                    xn4[0:32, ap, b0:b0 + 4, 1:17, 1:8],
                    xn4[64:96, ap, b0:b0 + 4, 1:17, 0:7])

    # Emission schedule: interleave layer-1 groups into the L0 tail (their
    # sources become ready after 1-3 L0 a-blocks), so the PE stays busy while
    # the L0 scatter (eviction-bound phase) drains.  Group g of layer 1 reads
    # sources 2g-1..2g+2, i.e. needs L0 a-blocks up to (2g+2)//4.
    # Chunk-level interleave of layer 1 into the L0 phase: a layer-1 chunk
    # (g, k) is inserted once every L0 chunk it reads (source a-blocks x
    # b-chunks) has been emitted for >= LAG further L0 chunks (so its scatter
    # has drained and the in-order PE queue never head-of-line blocks).
    LAG = 3
    ready_at = {}
    emitted = set()
    idx = 0
    for a0b in range(na4):
        for bc in range(8):
            emit_l0_chunk(a0b, bc)
            idx += 1
            for g in range(8):
                ablo = max(0, 2 * g - 1) // 4
                abhi = min(la - 1, 2 * g + 2) // 4
                for k in range(4):
                    if (g, k) in ready_at:
                        continue
                    need = [(ab, c) for ab in range(ablo, abhi + 1)
                            for c in range(max(0, 2 * k - 1),
                                           min(7, 2 * k + 2) + 1)]
                    if all(n[0] < a0b or (n[0] == a0b and n[1] <= bc)
                           for n in need):
                        ready_at[(g, k)] = idx
            # emit at most one lagged-ready layer-1 chunk per L0 chunk, and
            # none before idx 14 (the w1 weight DMA lands ~12us in)
            if idx >= 14:
                for (g, k), r in sorted(ready_at.items(), key=lambda kv: kv[1]):
                    if (g, k) not in emitted and idx - r >= LAG:
                        emit_mid_chunk(1, g, k)
                        emitted.add((g, k))
                        break
    for g in range(8):
        for k in range(4):
            if (g, k) not in emitted:
                emit_mid_chunk(1, g, k)
    for l in range(2, 5):
        for g in range(8):
            for k in range(4):
                emit_mid_chunk(l, g, k)

    # ================= layer 5 (a-banded, M=64 = 16a x 2o x 2t) =================
    # The last b-chunk is split in two independent psum groups so the first
    # half's eviction+DMA overlaps the second half's matmuls (shrinks the
    # end-of-program tail).
    xc4 = xview(bufs[0])
    for b0, bw in ((0, 4), (4, 4), (8, 4), (12, 2), (14, 2)):
        psf = pp.tile([128, 512], F32, tag="ps")
        ps = psf[0:64, 0:bw * 128]
        mms = []
        for gi, (db, dc) in enumerate(G9):
            for src in range(la):
                mms.append((gi, db, dc, src))
        nmm = len(mms)
        for idx, (gi, db, dc, src) in enumerate(mms):
            blo = max(b0, -db)
            bhi = min(b0 + bw, 16 - db)
            rhs = xc4[:, src, blo + db:bhi + db, dc + 1:dc + 17, :]
            lhsT = w5t[:, (gi * la + src) * 64:(gi * la + src) * 64 + 64]
            outv = ps[:, (blo - b0) * 128:(bhi - b0) * 128]
            nc.tensor.matmul(outv, lhsT, rhs,
                             start=(idx == 0), stop=(idx == nmm - 1))
        st = stg.tile([64, 512], F32, tag="stg")
        nc.vector.tensor_copy(st[:, 0:bw * 128], ps[:])
        nc.sync.dma_start(out_d[:, b0 * 128:(b0 + bw) * 128],
                          st[:, 0:bw * 128])


_CACHE = {}
LAST_RESULT = None


def _build(la, slopes):
    key = (la, tuple(slopes))
    if key not in _CACHE:
        nc = bacc.Bacc("TRN2")
        with tile.TileContext(nc) as tc:
            _conv_kernel(tc, la, slopes)
        nc.compile()
        _CACHE[key] = nc
    return _CACHE[key]


def kernel(x, k0, k1, k2, k3, k4, k5, slopes):
    x = np.asarray(x, np.float32)
    n, _, la = x.shape[:3]
    slopes_f = [float(s) for s in np.asarray(slopes, np.float32)]
    ws = _pack_weights((k0, k1, k2, k3, k4, k5), la)
    nc = _build(la, slopes_f)

    in_maps = []
    for i in range(n):
        m = {"xpad": _pack_x(x[i], la), "w0": ws[0], "w5": ws[5]}
        for l in (1, 2, 3, 4):
            m[f"w{l}"] = ws[l]
        in_maps.append(m)

    res = bass_utils.run_bass_kernel_spmd(nc, in_maps,
                                          core_ids=list(range(n)))
    global LAST_RESULT
    LAST_RESULT = res
    outs = []
    for i in range(n):
        od = res.results[i]["out_dec"].reshape(la, 2, 2, 16, 16, 8)
        # rows (a, o, t) x cols (b, c, j) -> [o, a, b, c, d = 2j + t]
        o = np.transpose(od, (1, 0, 3, 4, 5, 2)).reshape(2, la, 16, 16, 16)
        outs.append(o)
    return np.stack(outs).astype(np.float32)


# revision 7
# speedup vs baseline: 1.0511x; 1.0001x over previous
"""Trainium2 Bass kernel for a 6-layer 4D CNN (3^4 SAME convs + PReLU) — v2.

Problem: x (8, 2, 16,16,16,16) -> 6 conv layers, channels 2->32->32->32->32
->32->2, PReLU (scalar slope) after the first five convs.  Data-parallel over
batch N=8 across 8 cores.

v2 changes vs the v1 baseline (1824 us modeled):
  * Mid layers (1..4) use full M=128 matmuls: M = (2 a-outs x 2 d-outs x 32
    ch).  Each 2-wide a-output group accumulates streams from its <=4 source
    a-columns; the stationary for a source is a contiguous 128-col slice of a
    per-(db,dc) bank laid out [W(da=+1) | W(da=0) | W(da=-1)], so the
    (da_for_slot0, da_for_slot1) pair needed by each source pattern is one
    affine AP.  End sources use M=64 half matmuls (psum row offset).
    30 streams per (db,dc) pair per layer vs 46 at M=64: 812k -> 530k PE cyc.
  * Layer 5 uses an a-banded stationary: M = 64 = (16 a x 2 out-ch x 2
    d-outs); each source a streams once per (db,dc) into a +-1 band of psum
    rows (psum partition offset).  One [64,512] psum tile per b-chunk
    accumulates all 144 matmuls (zero-stationary matmul opens the group).
    812k/4-packed -> 283k PE cycles.
  * L0 input (d-shift replicated, padded, fp16) is packed on the host and
    DMA'd directly; the fp32 staging buffer and on-chip conversion are gone.
  * d-axis decimated activation layout X'[32*s + ch, (a, b, c_pad, j)] with
    d = 2*j + s - 1 is unchanged from v1, as are L0's a-partition scheme,
    PReLU eviction (ACT relu-scale + DVE scalar_tensor_tensor), and the
    j-shift copies that restore the redundant s=0/3 blocks.
"""

import sys

import numpy as np

for _p in ("/opt/trn_rl_repo", "/root/.axon_site/_ro/trn_rl_repo"):
    if _p not in sys.path:
        sys.path.append(_p)

import concourse.bass as bass  # noqa: E402
import concourse.mybir as mybir  # noqa: E402
import concourse.tile as tile  # noqa: E402
from concourse import bacc, bass_utils  # noqa: E402
from concourse._compat import with_exitstack  # noqa: E402

F32 = mybir.dt.float32
F16 = mybir.dt.float16

LB = 16
CP = 18   # padded c axis (c in -1..16)
DP = 18   # padded d axis in x_pad
J = 8     # d//2

# tap order: db=0 first so the first matmul of every psum group covers the
# full b-window (start=True zeroes the whole tile)
G9 = [(db, dc) for db in (0, -1, 1) for dc in (0, -1, 1)]
G_L0 = [(db, dc) for db in (0, -1, 1) for dc in (0, -1, 1)]


def _pack_weights(ks, la):
    """Host-side packing of conv kernels into stationary matrices (fp16)."""
    na4 = la // 4
    k0, k1, k2, k3, k4, k5 = [np.asarray(k, np.float32) for k in ks]

    # L0: W0[32*rg + i*la + a_in, (a0b*9 + g)*128 + a_j*32 + o]  (a0b-major
    # so the DMA can be sliced per a0b block)
    # K = 96 = 3 d-shift blocks (rg -> dd = rg - 1) x (2 ch x la a_in, banded)
    w0 = np.zeros((128, len(G_L0) * na4 * 128), np.float32)
    for gi, (db, dc) in enumerate(G_L0):
        for a0b in range(na4):
            cb = (a0b * len(G_L0) + gi) * 128
            for rg in range(3):
                for aj in range(4):
                    for da in (-1, 0, 1):
                        ain = a0b * 4 + aj + da
                        if not (0 <= ain < la):
                            continue
                        for i in range(2):
                            w0[32 * rg + i * la + ain,
                               cb + aj * 32:cb + aj * 32 + 32] = \
                                k0[:, i, da + 1, db + 1, dc + 1, rg]

    # mid layers: per (db,dc) bank of three da blocks ordered (+1, 0, -1);
    # block col = t*32 + o; W(da)[32*s + i, t*32 + o] = k[o,i,da,db,dc,s-t]
    def pack_mid(k):
        w = np.zeros((128, 9 * 3 * 64), np.float32)
        for gi, (db, dc) in enumerate(G9):
            for bi, da in enumerate((1, 0, -1)):
                cb = (gi * 3 + bi) * 64
                for s in range(4):
                    for t in range(2):
                        if 0 <= s - t <= 2:
                            w[32 * s:32 * s + 32,
                              cb + t * 32:cb + t * 32 + 32] = \
                                k[:, :, da + 1, db + 1, dc + 1, s - t].T
        return w

    # L5 banded, full-M stationaries (psum row offsets must be 32-aligned):
    # per (gi, src) a [128, 64] matrix, col = 4*a' + 2*o + t nonzero only for
    # a' in the +-1 band of src (da = src - a').
    w5 = np.zeros((128, 9 * la * 64), np.float32)
    for gi, (db, dc) in enumerate(G9):
        for src in range(la):
            cb = (gi * la + src) * 64
            for ap_ in (src - 1, src, src + 1):
                if not (0 <= ap_ < la):
                    continue
                da = src - ap_
                for s in range(4):
                    for t in range(2):
                        if 0 <= s - t <= 2:
                            for o in range(2):
                                w5[32 * s:32 * s + 32,
                                   cb + 4 * ap_ + 2 * o + t] = \
                                    k5[o, :, da + 1, db + 1, dc + 1, s - t]

    return ([w0.astype(np.float16)] +
            [pack_mid(k).astype(np.float16) for k in (k1, k2, k3, k4)] +
            [w5.astype(np.float16)])


def _pack_x(x1, la):
    """One sample (2, la, 16, 16, 16) -> padded fp16 [128, 16*18*18].

    partition = 32*rg + i*la + a holds x[i, a] shifted in d by dd = rg - 1;
    b unpadded, c padded to 18 (data at 1..16), d padded to 18.
    """
    xp = np.zeros((128, LB, CP, DP), np.float16)
    x1 = np.asarray(x1, np.float16)
    for rg in range(3):
        for i in range(2):
            xp[32 * rg + i * la:32 * rg + i * la + la, :, 1:17,
               2 - rg:18 - rg] = x1[i]
    return np.ascontiguousarray(xp.reshape(128, LB * CP * DP))


@with_exitstack
def _conv_kernel(ctx, tc, la, slopes):
    """Emit the full 6-layer conv program. slopes: python floats len 5."""
    nc = tc.nc
    na4 = la // 4
    xcols = la * LB * CP * J

    xpad_d = nc.dram_tensor("xpad", [128, LB * CP * DP], F16,
                            kind="ExternalInput")
    w0_d = nc.dram_tensor("w0", [128, len(G_L0) * na4 * 128],
                          F16, kind="ExternalInput")
    wmid_d = [nc.dram_tensor(f"w{l}", [128, 9 * 3 * 64], F16,
                             kind="ExternalInput") for l in (1, 2, 3, 4)]
    w5_d = nc.dram_tensor("w5", [128, 9 * la * 64], F16, kind="ExternalInput")
    out_d = nc.dram_tensor("out_dec", [64, la * 128], F32,
                           kind="ExternalOutput")

    const = ctx.enter_context(tc.tile_pool(name="const", bufs=1))
    pp = ctx.enter_context(tc.tile_pool(name="ps", bufs=8, space="PSUM"))
    stg = ctx.enter_context(tc.tile_pool(name="stg", bufs=4))

    # ---- load weights + padded input ----
    # xpad and w0 are DMA'd in b-/a0b-slices so layer 0's first chunks are
    # gated by ~0.6MB of DMA instead of the full 2.5MB.
    xpad = const.tile([128, LB * CP * DP], F16)
    xpv = xpad.rearrange("p (b cd) -> p b cd", b=LB, cd=CP * DP)
    xdv = xpad_d.rearrange("p (b cd) -> p b cd", b=LB, cd=CP * DP)
    w0t = const.tile([128, w0_d.shape[1]], F16)
    # interleave xpad/w0 slices so the first L0 chunk is gated by only the
    # first xpad b-slice + first w0 block (~0.6MB), not the whole 2.5MB
    for a0b in range(na4):
        b0 = a0b * 4
        nc.sync.dma_start(xpv[:, b0:b0 + 4, :], xdv[:, b0:b0 + 4, :])
        cb = a0b * 9 * 128
        nc.sync.dma_start(w0t[:, cb:cb + 9 * 128], w0_d[:, cb:cb + 9 * 128])
    wmt = []
    for wd in wmid_d:
        t = const.tile([128, 9 * 3 * 64], F16, tag=wd.name)
        nc.sync.dma_start(t[:], wd[:])
        wmt.append(t)
    w5t = const.tile([128, 9 * la * 64], F16)
    nc.sync.dma_start(w5t[:], w5_d[:])

    xp4 = xpad.rearrange("p (b c d) -> p b c d", b=LB, c=CP, d=DP)

    # ---- X' ping-pong buffers ----
    xa = const.tile([128, xcols], F16)
    xb = const.tile([128, xcols], F16)
    bufs = [xa, xb]

    def xview(t):
        return t.rearrange("p (a b c j) -> p a b c j", a=la, b=LB, c=CP, j=J)

    # scatter map: s -> (j_lo, j_cnt, d_lo)  [d = 2j + s - 1]
    SMAP = {0: (1, 7, 1), 1: (0, 8, 0), 2: (0, 8, 1), 3: (0, 7, 2)}

    # ---- PE warm-up ----
    # Zero-matmuls issued while the first xpad/w0 DMA slices land: they keep
    # the tensor engine busy from t~0.1us so the p-state ramp (first 3us of a
    # continuous run at half clock) is spent inside the DMA shadow, and the
    # first real matmul starts at full speed with no idle gap.
    wz = const.tile([128, 128], F16)
    nc.vector.memset(wz[:], 0.0)
    wps = pp.tile([128, 512], F32, tag="ps")
    for _ in range(40):
        nc.tensor.matmul(wps[:, 0:128], wz[:], wz[:], start=True, stop=True)

    # Only the padding regions need zeroing (evictions write everything else
    # before it is read): c = 0 / 17 columns, plus the j-edge zeros of the
    # redundant d blocks (s=0 @ j=0 -> d=-1, s=3 @ j=7 -> d=16).  These run
    # on Pool during the L0 phase; the first consumer is layer 1's rhs.
    for t in bufs:
        v = xview(t)
        nc.gpsimd.memset(v[:, :, :, 0:1, :], 0.0)
        nc.gpsimd.memset(v[:, :, :, 17:18, :], 0.0)
        nc.gpsimd.memset(v[0:32, :, :, 1:17, 0:1], 0.0)
        nc.gpsimd.memset(v[96:128, :, :, 1:17, 7:8], 0.0)

    # ================= layer 0 =================
    def emit_l0_chunk(a0b, bc):
        xn4 = xview(bufs[0])
        a_slope = slopes[0]
        if True:
            b0 = bc * 2
            ps = pp.tile([128, 512], F32, tag="ps")
            p4 = ps.rearrange("p (b c d) -> p b c d", b=2, c=16, d=16)
            for gi, (db, dc) in enumerate(G_L0):
                blo = max(b0, -db)
                bhi = min(b0 + 2, 16 - db)
                cb = (a0b * len(G_L0) + gi) * 128
                # K = 96: three d-shift blocks; rhs d-slice 1:17 uniform
                rhs = xp4[0:96, blo + db:bhi + db, dc + 1:dc + 17, 1:17]
                out = p4[:, blo - b0:bhi - b0, :, :]
                nc.tensor.matmul(out, w0t[0:96, cb:cb + 128], rhs,
                                 start=(gi == 0),
                                 stop=(gi == len(G_L0) - 1))
            # prelu in ONE ACT op (Lrelu, alpha=slope), then scatter the
            # chunk across DVE / GPSIMD / ACT so the scatter keeps up with
            # the PE during the L0 phase.
            sg = stg.tile([128, 512], F16, tag="l0st")
            nc.scalar.activation(sg[:], ps[:],
                                 mybir.ActivationFunctionType.Lrelu,
                                 alpha=a_slope)
            sg4 = sg.rearrange("p (b c d) -> p b c d", b=2, c=16, d=16)
            eng = 0
            for aj in range(4):
                a = a0b * 4 + aj
                for s in range(4):
                    jlo, jcnt, dlo = SMAP[s]
                    dst = xn4[32 * s:32 * s + 32, a, b0:b0 + 2, 1:17,
                              jlo:jlo + jcnt]
                    src = sg4[32 * aj:32 * aj + 32, :, :,
                              dlo:dlo + 2 * jcnt - 1:2]
                    if eng in (0, 3):
                        nc.vector.tensor_copy(dst, src)
                    elif eng in (1, 4):
                        nc.gpsimd.tensor_copy(dst, src)
                    else:
                        nc.scalar.copy(dst, src)
                    eng = (eng + 1) % 5

    # ================= layers 1..4 =================
    # M = 128 = (2 a-outs x 2 t x 32 ch); per (db,dc) the stationary for
    # source pattern p (src = a0 - 1 + p) is the contiguous da-block pair
    # (2-p, 3-p) of the bank [W(+1) | W(0) | W(-1)]; p=0/3 are M=64 halves.
    def emit_mid_chunk(l, g, chunk):
        xc4 = xview(bufs[(l + 1) % 2])
        xn4 = xview(bufs[l % 2])
        wt = wmt[l - 1]
        a_slope = slopes[l]
        a0 = 2 * g
        if True:
            b0 = 4 * chunk
            ps = pp.tile([128, 512], F32, tag="ps")
            mms = []
            for gi, (db, dc) in enumerate(G9):
                for p in (1, 2, 0, 3):   # full-M patterns first
                    src = a0 - 1 + p
                    if not (0 <= src < la):
                        continue
                    mms.append((gi, db, dc, p, src))
            nmm = len(mms)
            for idx, (gi, db, dc, p, src) in enumerate(mms):
                blo = max(b0, -db)
                bhi = min(b0 + 4, 16 - db)
                rhs = xc4[:, src, blo + db:bhi + db, dc + 1:dc + 17, :]
                cw = (blo - b0) * 128, (bhi - b0) * 128
                if p in (1, 2):
                    lhsT = wt[:, (gi * 3 + 2 - p) * 64:
                              (gi * 3 + 4 - p) * 64]
                    outv = ps[:, cw[0]:cw[1]]
                elif p == 0:   # only slot q=0 (out a0), da = -1
                    lhsT = wt[:, (gi * 3 + 2) * 64:(gi * 3 + 3) * 64]
                    outv = ps[0:64, cw[0]:cw[1]]
                else:          # p == 3: only slot q=1 (out a0+1), da = +1
                    lhsT = wt[:, (gi * 3 + 0) * 64:(gi * 3 + 1) * 64]
                    outv = ps[64:128, cw[0]:cw[1]]
                nc.tensor.matmul(outv, lhsT, rhs,
                                 start=(idx == 0), stop=(idx == nmm - 1))
            # prelu evictions: psum row block (64q + 32t) -> s' = t + 1;
            # one Lrelu ACT op per block; j-shift copies on DVE/GPSIMD
            for q in (0, 1):
                ap = a0 + q
                for t in (0, 1):
                    dst = xn4[32 * (t + 1):32 * (t + 2), ap,
                              b0:b0 + 4, 1:17, 0:8]
                    srcv = ps[64 * q + 32 * t:64 * q + 32 * t + 32, :]
                    nc.scalar.activation(
                        dst, srcv, mybir.ActivationFunctionType.Lrelu,
                        alpha=a_slope)
                # j-shift copies: s'=3 <- s'=1 (j+1) ; s'=0 <- s'=2 (j-1)
                nc.vector.tensor_copy(
                    xn4[96:128, ap, b0:b0 + 4, 1:17, 0:7],
                    xn4[32:64, ap, b0:b0 + 4, 1:17, 1:8])
                nc.gpsimd.tensor_copy(
                    xn4[0:32, ap, b0:b0 + 4, 1:17, 1:8],
                    xn4[64:96, ap, b0:b0 + 4, 1:17, 0:7])

    # Emission schedule: interleave layer-1 groups into the L0 tail (their
    # sources become ready after 1-3 L0 a-blocks), so the PE stays busy while
    # the L0 scatter (eviction-bound phase) drains.  Group g of layer 1 reads
    # sources 2g-1..2g+2, i.e. needs L0 a-blocks up to (2g+2)//4.
    # Chunk-level interleave of layer 1 into the L0 phase: a layer-1 chunk
    # (g, k) is inserted once every L0 chunk it reads (source a-blocks x
    # b-chunks) has been emitted for >= LAG further L0 chunks (so its scatter
    # has drained and the in-order PE queue never head-of-line blocks).
    LAG = 3
    ready_at = {}
    emitted = set()
    idx = 0
    for a0b in range(na4):
        for bc in range(8):
            emit_l0_chunk(a0b, bc)
            idx += 1
            for g in range(8):
                ablo = max(0, 2 * g - 1) // 4
                abhi = min(la - 1, 2 * g + 2) // 4
                for k in range(4):
                    if (g, k) in ready_at:
                        continue
                    need = [(ab, c) for ab in range(ablo, abhi + 1)
                            for c in range(max(0, 2 * k - 1),
                                           min(7, 2 * k + 2) + 1)]
                    if all(n[0] < a0b or (n[0] == a0b and n[1] <= bc)
                           for n in need):
                        ready_at[(g, k)] = idx
            # emit at most one lagged-ready layer-1 chunk per L0 chunk, and
            # none before idx 14 (the w1 weight DMA lands ~12us in)
            if idx >= 14:
                for (g, k), r in sorted(ready_at.items(), key=lambda kv: kv[1]):
                    if (g, k) not in emitted and idx - r >= LAG:
                        emit_mid_chunk(1, g, k)
                        emitted.add((g, k))
                        break
    for g in range(8):
        for k in range(4):
            if (g, k) not in emitted:
                emit_mid_chunk(1, g, k)
    for l in range(2, 5):
        for g in range(8):
            for k in range(4):
                emit_mid_chunk(l, g, k)

    # ================= layer 5 (a-banded, M=64 = 16a x 2o x 2t) =================
    # The last b-chunk is split in two independent psum groups so the first
    # half's eviction+DMA overlaps the second half's matmuls (shrinks the
    # end-of-program tail).
    xc4 = xview(bufs[0])
    for b0, bw in ((0, 4), (4, 4), (8, 4), (12, 2), (14, 1), (15, 1)):
        psf = pp.tile([128, 512], F32, tag="ps")
        ps = psf[0:64, 0:bw * 128]
        mms = []
        for gi, (db, dc) in enumerate(G9):
            for src in range(la):
                if max(b0, -db) < min(b0 + bw, 16 - db):  # non-empty window
                    mms.append((gi, db, dc, src))
        nmm = len(mms)
        for idx, (gi, db, dc, src) in enumerate(mms):
            blo = max(b0, -db)
            bhi = min(b0 + bw, 16 - db)
            rhs = xc4[:, src, blo + db:bhi + db, dc + 1:dc + 17, :]
            lhsT = w5t[:, (gi * la + src) * 64:(gi * la + src) * 64 + 64]
            outv = ps[:, (blo - b0) * 128:(bhi - b0) * 128]
            nc.tensor.matmul(outv, lhsT, rhs,
                             start=(idx == 0), stop=(idx == nmm - 1))
        st = stg.tile([64, 512], F32, tag="stg")
        nc.vector.tensor_copy(st[:, 0:bw * 128], ps[:])
        nc.sync.dma_start(out_d[:, b0 * 128:(b0 + bw) * 128],
                          st[:, 0:bw * 128])


_CACHE = {}
LAST_RESULT = None


def _build(la, slopes):
    key = (la, tuple(slopes))
    if key not in _CACHE:
        nc = bacc.Bacc("TRN2")
        with tile.TileContext(nc) as tc:
            _conv_kernel(tc, la, slopes)
        nc.compile()
        _CACHE[key] = nc
    return _CACHE[key]


def kernel(x, k0, k1, k2, k3, k4, k5, slopes):
    x = np.asarray(x, np.float32)
    n, _, la = x.shape[:3]
    slopes_f = [float(s) for s in np.asarray(slopes, np.float32)]
    ws = _pack_weights((k0, k1, k2, k3, k4, k5), la)
    nc = _build(la, slopes_f)

    in_maps = []
    for i in range(n):
        m = {"xpad": _pack_x(x[i], la), "w0": ws[0], "w5": ws[5]}
        for l in (1, 2, 3, 4):
            m[f"w{l}"] = ws[l]
        in_maps.append(m)

    res = bass_utils.run_bass_kernel_spmd(nc, in_maps,
                                          core_ids=list(range(n)))
    global LAST_RESULT
    LAST_RESULT = res
    outs = []
    for i in range(n):
        od = res.results[i]["out_dec"].reshape(la, 2, 2, 16, 16, 8)
        # rows (a, o, t) x cols (b, c, j) -> [o, a, b, c, d = 2j + t]
        o = np.transpose(od, (1, 0, 3, 4, 5, 2)).reshape(2, la, 16, 16, 16)
        outs.append(o)
    return np.stack(outs).astype(np.float32)
